# revision 1
# baseline (speedup 1.0000x reference)
import sys, os
sys.path.insert(0, "/opt/trn_rl_repo")
import numpy as np
import ml_dtypes
_bf16np = ml_dtypes.bfloat16

from contextlib import ExitStack
import concourse.tile as tile
from concourse import bass, bacc, mybir
from concourse.bass import IndirectOffsetOnAxis
from concourse.bass_utils import run_bass_kernel_spmd

N = 50000
P = 128
NCORES = 8
NPC = N // NCORES          # 6250 nodes per core
TPC = (NPC + P - 1) // P   # 49 node tiles per core
NPAD = TPC * P             # 6272 padded nodes per core
NFULL = NCORES * NPAD      # 50176 padded gather-source rows
D = 128
DOUT = 40

_cache = {}


def _build(K, T):
    nc = bacc.Bacc("TRN2", target_bir_lowering=False, debug=False,
                   num_devices=NCORES)
    f32, i32, bf16 = mybir.dt.float32, mybir.dt.int32, mybir.dt.bfloat16

    feat = nc.dram_tensor("feat", [NPAD, D], bf16, kind="ExternalInput").ap()
    srcd = nc.dram_tensor("srcd", [P, T], i32, kind="ExternalInput").ap()
    dstd = nc.dram_tensor("dstd", [P, T], f32, kind="ExternalInput").ap()
    normd = nc.dram_tensor("normd", [P, TPC], f32, kind="ExternalInput").ap()
    iotad = nc.dram_tensor("iotad", [P, P], f32, kind="ExternalInput").ap()
    wd = [nc.dram_tensor(f"w{i}", [D, D if i < 3 else DOUT], f32,
                         kind="ExternalInput").ap() for i in range(4)]
    outd = nc.dram_tensor("out", [NPAD, DOUT], f32, kind="ExternalOutput").ap()

    with tile.TileContext(nc) as tc, ExitStack() as ctx:
        dram = ctx.enter_context(tc.tile_pool(name="dram", bufs=2, space="DRAM"))
        consts = ctx.enter_context(tc.tile_pool(name="consts", bufs=1))
        hpool = ctx.enter_context(tc.tile_pool(name="hs", bufs=2))
        msgp = ctx.enter_context(tc.tile_pool(name="msg", bufs=24))
        selp = ctx.enter_context(tc.tile_pool(name="sel", bufs=24))
        aggp = ctx.enter_context(tc.tile_pool(name="agg", bufs=4))
        outp = ctx.enter_context(tc.tile_pool(name="outp", bufs=4))
        ps1 = ctx.enter_context(tc.tile_pool(name="ps1", bufs=3, space="PSUM"))
        ps2 = ctx.enter_context(tc.tile_pool(name="ps2", bufs=3, space="PSUM"))

        src_sb = consts.tile([P, T], i32)
        nc.gpsimd.dma_start(src_sb[:], srcd[:])
        dst_sb = consts.tile([P, T], f32)
        nc.gpsimd.dma_start(dst_sb[:], dstd[:])
        norm_sb = consts.tile([P, TPC], f32)
        nc.gpsimd.dma_start(norm_sb[:], normd[:])
        iota_sb = consts.tile([P, P], f32)
        nc.gpsimd.dma_start(iota_sb[:], iotad[:])
        w_sb = []
        for i in range(4):
            dcol = D if i < 3 else DOUT
            w = consts.tile([P, dcol], f32)
            nc.gpsimd.dma_start(w[:], wd[i][:])
            w_sb.append(w)

        h_scaled = None
        for layer in range(4):
            dcol = D if layer < 3 else DOUT
            bounce = dram.tile([NPAD, D], bf16)
            hfull = dram.tile([NFULL, D], bf16)
            if layer == 0:
                for t in range(TPC):
                    ft = msgp.tile([P, D], bf16)
                    nc.gpsimd.dma_start(ft[:], feat[t * P:(t + 1) * P, :])
                    nc.gpsimd.dma_start(bounce[t * P:(t + 1) * P, :], ft[:])
            else:
                for t in range(TPC):
                    nc.gpsimd.dma_start(bounce[t * P:(t + 1) * P, :],
                                        h_scaled[:, t * P:(t + 1) * P])
            nc.gpsimd.collective_compute(
                "AllGather", mybir.AluOpType.bypass,
                replica_groups=[list(range(NCORES))],
                ins=[bounce.opt()], outs=[hfull.opt()],
            )
            if layer < 3:
                h_next = hpool.tile([P, NPAD], bf16)
            e = 0
            for t in range(TPC):
                psA = ps1.tile([P, P], f32, space="PSUM")
                kt = K[t]
                for k in range(kt):
                    msg = msgp.tile([P, D], bf16)
                    nc.gpsimd.indirect_dma_start(
                        out=msg[:], out_offset=None, in_=hfull[:],
                        in_offset=IndirectOffsetOnAxis(ap=src_sb[:, e:e + 1],
                                                       axis=0))
                    sel = selp.tile([P, P], bf16)
                    nc.vector.tensor_tensor(
                        out=sel[:],
                        in0=dst_sb[:, e:e + 1].to_broadcast([P, P]),
                        in1=iota_sb[:], op=mybir.AluOpType.is_equal)
                    nc.tensor.matmul(out=psA[:], lhsT=msg[:], rhs=sel[:],
                                     start=(k == 0), stop=(k == kt - 1))
                    e += 1
                aggT = aggp.tile([P, P], f32)
                nc.vector.tensor_copy(aggT[:], psA[:])
                psO = ps2.tile([P, dcol], f32, space="PSUM")
                nc.tensor.matmul(out=psO[:], lhsT=aggT[:],
                                 rhs=w_sb[layer][:, :dcol],
                                 start=True, stop=True)
                if layer < 3:
                    tmp = outp.tile([P, D], f32)
                    nc.scalar.activation(
                        out=tmp[:], in_=psO[:],
                        func=mybir.ActivationFunctionType.Relu,
                        scale=norm_sb[:, t:t + 1])
                    nc.vector.tensor_tensor(
                        out=h_next[:, t * P:(t + 1) * P], in0=tmp[:],
                        in1=norm_sb[:, t:t + 1].to_broadcast([P, P]),
                        op=mybir.AluOpType.mult)
                else:
                    ot = outp.tile([P, DOUT], f32)
                    nc.scalar.activation(
                        out=ot[:], in_=psO[:],
                        func=mybir.ActivationFunctionType.Copy,
                        scale=norm_sb[:, t:t + 1])
                    nc.gpsimd.dma_start(outd[t * P:(t + 1) * P, :], ot[:])
            if layer < 3:
                h_scaled = h_next
    nc.compile()
    return nc


def kernel(features, edge_index, W0, W1, W2, W3):
    features = np.asarray(features, dtype=np.float32)
    src = np.asarray(edge_index[0], dtype=np.int64)
    dst = np.asarray(edge_index[1], dtype=np.int64)
    Ws = [np.ascontiguousarray(np.asarray(w, dtype=np.float32))
          for w in (W0, W1, W2, W3)]

    deg = np.bincount(dst, minlength=N).astype(np.float32)
    norm = 1.0 / np.sqrt(np.maximum(deg, 1.0))

    # per-core edge partition by dst range; group edges by dst node-tile
    per_core = []
    cnt = np.zeros((NCORES, TPC), dtype=np.int64)
    for c in range(NCORES):
        m = (dst >= c * NPC) & (dst < (c + 1) * NPC)
        es = src[m]
        ed = dst[m] - c * NPC
        order = np.argsort(ed, kind="stable")
        es, ed = es[order], ed[order]
        tt = ed // P
        for t in range(TPC):
            cnt[c, t] = np.count_nonzero(tt == t)
        per_core.append((es, ed, tt))
    K = [max(1, int(-(-cnt[:, t].max() // P))) for t in range(TPC)]
    T = int(sum(K))

    in_maps = []
    iota = np.tile(np.arange(P, dtype=np.float32), (P, 1))
    for c in range(NCORES):
        es, ed, tt = per_core[c]
        src_col = np.zeros((T, P), dtype=np.int32)
        dst_col = np.full((T, P), -1.0, dtype=np.float32)
        col = 0
        for t in range(TPC):
            sel = tt == t
            s_t = es[sel]
            d_t = ed[sel] - t * P
            n = len(s_t)
            gidx = (s_t // NPC) * NPAD + (s_t % NPC)
            buf_s = np.zeros(K[t] * P, dtype=np.int32)
            buf_d = np.full(K[t] * P, -1.0, dtype=np.float32)
            buf_s[:n] = gidx
            buf_d[:n] = d_t.astype(np.float32)
            src_col[col:col + K[t]] = buf_s.reshape(K[t], P)
            dst_col[col:col + K[t]] = buf_d.reshape(K[t], P)
            col += K[t]
        nloc = np.zeros(NPAD, dtype=np.float32)
        nloc[:NPC] = norm[c * NPC:(c + 1) * NPC]
        feat_s = np.zeros((NPAD, D), dtype=np.float32)
        feat_s[:NPC] = features[c * NPC:(c + 1) * NPC] * nloc[:NPC, None]
        in_maps.append({
            "feat": feat_s.astype(_bf16np),
            "srcd": np.ascontiguousarray(src_col.T),
            "dstd": np.ascontiguousarray(dst_col.T),
            "normd": np.ascontiguousarray(nloc.reshape(TPC, P).T),
            "iotad": iota,
            "w0": Ws[0], "w1": Ws[1], "w2": Ws[2], "w3": Ws[3],
        })

    key = (tuple(K),)
    if key not in _cache:
        _cache[key] = _build(K, T)
    nc = _cache[key]
    global _last_in_maps
    _last_in_maps = in_maps
    res = run_bass_kernel_spmd(nc, in_maps, list(range(NCORES)))
    out = np.concatenate([res.results[c]["out"][:NPC] for c in range(NCORES)],
                         axis=0)
    return out.astype(np.float32)



# revision 5
# speedup vs baseline: 328.7403x; 328.7403x over previous
import sys
sys.path.insert(0, "/opt/trn_rl_repo")
import numpy as np
import ml_dtypes

_bf16np = ml_dtypes.bfloat16

from contextlib import ExitStack
import concourse.tile as tile
from concourse import bass, bacc, mybir
from concourse.library_config import mlp

N = 50000
P = 128
NCORES = 8
NPC = N // NCORES            # 6250 nodes per core
TPC = (NPC + P - 1) // P     # 49 node tiles per core
NPAD = TPC * P               # 6272 padded nodes per core
NFULL = NCORES * NPAD        # 50176 padded gather-source rows
HALF = NFULL // 2            # 25088 (< 2**15, fits int16 gather indices)
D = 128
DOUT = 40
CHUNK = 7                    # dst tiles per gather chunk

_cache = {}
_runner_cache = {}
_last_in_maps = None


def _build(KA, KB):
    """KA/KB: per-dst-tile slot counts (128 edges per slot) for gather halves
    A (src row < HALF) and B (src row >= HALF), shared across all cores."""
    nc = bacc.Bacc("TRN2", target_bir_lowering=False, debug=False,
                   num_devices=NCORES)
    f32, i16, bf16 = mybir.dt.float32, mybir.dt.int16, mybir.dt.bfloat16

    TA, TB = sum(KA), sum(KB)
    feat = nc.dram_tensor("feat", [NPAD, D], bf16, kind="ExternalInput").ap()
    idxd = nc.dram_tensor("idxd", [P, (TA + TB) * 8], i16,
                          kind="ExternalInput").ap()
    dstd = nc.dram_tensor("dstd", [P, TA + TB], bf16,
                          kind="ExternalInput").ap()
    norm2d = nc.dram_tensor("norm2d", [P, TPC], f32, kind="ExternalInput").ap()
    norm1d = nc.dram_tensor("norm1d", [P, TPC], f32, kind="ExternalInput").ap()
    iotad = nc.dram_tensor("iotad", [P, P], bf16, kind="ExternalInput").ap()
    wd = [nc.dram_tensor(f"w{i}", [D, D if i < 3 else DOUT], bf16,
                         kind="ExternalInput").ap() for i in range(4)]
    outd = nc.dram_tensor("out", [NPAD, DOUT], bf16, kind="ExternalOutput").ap()

    # chunk boundaries over dst tiles, with per-chunk slot-column offsets
    chunks = []
    offA = offB = 0
    for c0 in range(0, TPC, CHUNK):
        ts = list(range(c0, min(c0 + CHUNK, TPC)))
        ka = sum(KA[t] for t in ts)
        kb = sum(KB[t] for t in ts)
        chunks.append((ts, offA, ka, offB, kb))
        offA += ka
        offB += kb

    with tile.TileContext(nc) as tc, ExitStack() as ctx:
        nc.gpsimd.load_library(mlp)
        dram = ctx.enter_context(tc.tile_pool(name="dram", bufs=2,
                                              space="DRAM"))
        shp = ctx.enter_context(tc.tile_pool(name="shp", bufs=2,
                                             space="DRAM"))
        consts = ctx.enter_context(tc.tile_pool(name="consts", bufs=1))
        hpool = ctx.enter_context(tc.tile_pool(name="hs", bufs=2))
        msgpA = ctx.enter_context(tc.tile_pool(name="msgA", bufs=2))
        msgpB = ctx.enter_context(tc.tile_pool(name="msgB", bufs=2))
        selpA = ctx.enter_context(tc.tile_pool(name="selA", bufs=2))
        selpB = ctx.enter_context(tc.tile_pool(name="selB", bufs=2))
        aggp = ctx.enter_context(tc.tile_pool(name="agg", bufs=4))
        outp = ctx.enter_context(tc.tile_pool(name="outp", bufs=1))
        ps1 = ctx.enter_context(tc.tile_pool(name="ps1", bufs=3, space="PSUM"))
        ps2 = ctx.enter_context(tc.tile_pool(name="ps2", bufs=3, space="PSUM"))

        idx_sb = consts.tile([P, (TA + TB) * 8], i16)
        nc.sync.dma_start(idx_sb[:], idxd[:])
        dst_sb = consts.tile([P, TA + TB], bf16)
        nc.sync.dma_start(dst_sb[:], dstd[:])
        norm2_sb = consts.tile([P, TPC], f32)
        nc.sync.dma_start(norm2_sb[:], norm2d[:])
        norm1_sb = consts.tile([P, TPC], f32)
        nc.sync.dma_start(norm1_sb[:], norm1d[:])
        iota_sb = consts.tile([P, P], bf16)
        nc.sync.dma_start(iota_sb[:], iotad[:])
        w_sb = []
        for i in range(4):
            dcol = D if i < 3 else DOUT
            w = consts.tile([P, dcol], bf16)
            nc.sync.dma_start(w[:], wd[i][:])
            w_sb.append(w)
        out_sb = outp.tile([P, TPC * DOUT], bf16)

        feat_int = dram.tile([NPAD, D], bf16)
        nc.sync.dma_start(feat_int[:], feat[:])

        h_prev = None
        for layer in range(4):
            dcol = D if layer < 3 else DOUT
            hfull = shp.tile([NFULL, D], bf16, addr_space="Shared")
            src_ag = feat_int if layer == 0 else h_prev
            nc.gpsimd.collective_compute(
                "AllGather", mybir.AluOpType.bypass,
                replica_groups=[list(range(NCORES))],
                ins=[src_ag[:]], outs=[hfull[:]],
            )
            if layer < 3:
                h_next = hpool.tile([P, TPC * D], bf16)
            for ts, oA, ka, oB, kb in chunks:
                if ka:
                    msgA = msgpA.tile([P, ka, D], bf16)
                    for s0 in range(0, ka, 8):
                        sw = min(8, ka - s0)
                        nc.gpsimd.dma_gather(
                            msgA[:, s0:s0 + sw, :], hfull[0:HALF, :],
                            idx_sb[:, (oA + s0) * 8:(oA + s0 + sw) * 8],
                            sw * P, sw * P, D)
                    selA = selpA.tile([P, ka * P], bf16)
                    nc.vector.tensor_tensor(
                        out=selA[:].rearrange("p (n r) -> p n r", n=ka),
                        in0=dst_sb[:, oA:oA + ka].unsqueeze(2)
                            .broadcast_to([P, ka, P]),
                        in1=iota_sb[:].unsqueeze(1).broadcast_to([P, ka, P]),
                        op=mybir.AluOpType.is_equal)
                if kb:
                    msgB = msgpB.tile([P, kb, D], bf16)
                    for s0 in range(0, kb, 8):
                        sw = min(8, kb - s0)
                        nc.gpsimd.dma_gather(
                            msgB[:, s0:s0 + sw, :], hfull[HALF:NFULL, :],
                            idx_sb[:, (TA + oB + s0) * 8:(TA + oB + s0 + sw) * 8],
                            sw * P, sw * P, D)
                    selB = selpB.tile([P, kb * P], bf16)
                    nc.vector.tensor_tensor(
                        out=selB[:].rearrange("p (n r) -> p n r", n=kb),
                        in0=dst_sb[:, TA + oB:TA + oB + kb].unsqueeze(2)
                            .broadcast_to([P, kb, P]),
                        in1=iota_sb[:].unsqueeze(1).broadcast_to([P, kb, P]),
                        op=mybir.AluOpType.is_equal)
                jA = jB = 0
                for t in ts:
                    nslots = KA[t] + KB[t]
                    if nslots == 0:
                        if layer < 3:
                            nc.vector.memset(h_next[:, t * D:(t + 1) * D], 0.0)
                        else:
                            nc.vector.memset(
                                out_sb[:, t * DOUT:(t + 1) * DOUT], 0.0)
                        continue
                    psA = ps1.tile([P, P], f32, space="PSUM")
                    k = 0
                    for _ in range(KA[t]):
                        nc.tensor.matmul(
                            out=psA[:], lhsT=msgA[:, jA, :],
                            rhs=selA[:, jA * P:(jA + 1) * P],
                            start=(k == 0), stop=(k == nslots - 1))
                        jA += 1
                        k += 1
                    for _ in range(KB[t]):
                        nc.tensor.matmul(
                            out=psA[:], lhsT=msgB[:, jB, :],
                            rhs=selB[:, jB * P:(jB + 1) * P],
                            start=(k == 0), stop=(k == nslots - 1))
                        jB += 1
                        k += 1
                    aggT = aggp.tile([P, P], bf16)
                    nc.vector.tensor_copy(aggT[:], psA[:])
                    psO = ps2.tile([P, dcol], f32, space="PSUM")
                    nc.tensor.matmul(out=psO[:], lhsT=aggT[:],
                                     rhs=w_sb[layer][:, :dcol],
                                     start=True, stop=True)
                    if layer < 3:
                        nc.scalar.activation(
                            out=h_next[:, t * D:(t + 1) * D], in_=psO[:],
                            func=mybir.ActivationFunctionType.Relu,
                            scale=norm2_sb[:, t:t + 1])
                    else:
                        nc.scalar.activation(
                            out=out_sb[:, t * DOUT:(t + 1) * DOUT], in_=psO[:],
                            func=mybir.ActivationFunctionType.Copy,
                            scale=norm1_sb[:, t:t + 1])
            if layer < 3:
                bounce = dram.tile([NPAD, D], bf16)
                nc.sync.dma_start(
                    bounce[:].rearrange("(t p) f -> p t f", p=P),
                    h_next[:].rearrange("p (t f) -> p t f", t=TPC))
                h_prev = bounce
        nc.sync.dma_start(
            outd[:].rearrange("(t p) f -> p t f", p=P),
            out_sb[:].rearrange("p (t f) -> p t f", t=TPC))
    nc.compile()
    return nc


def _preprocess(features, edge_index, Ws):
    src = np.asarray(edge_index[0], dtype=np.int64)
    dst = np.asarray(edge_index[1], dtype=np.int64)
    features = np.asarray(features, dtype=np.float32)

    deg = np.bincount(dst, minlength=N).astype(np.float32)
    norm = 1.0 / np.sqrt(np.maximum(deg, 1.0))

    core = dst // NPC
    ld = dst - core * NPC
    tt = ld >> 7
    rr = ld & 127
    gidx = (src // NPC) * NPAD + (src % NPC)
    halfB = (gidx >= HALF).astype(np.int64)

    key = (core * 2 + halfB) * TPC + tt
    cnt = np.bincount(key, minlength=NCORES * 2 * TPC).reshape(NCORES, 2, TPC)
    KA = [int(x) for x in -(-cnt[:, 0, :].max(axis=0) // P)]
    KB = [int(x) for x in -(-cnt[:, 1, :].max(axis=0) // P)]
    TA, TB = sum(KA), sum(KB)
    baseA = P * np.concatenate([[0], np.cumsum(KA)]).astype(np.int64)
    baseB = P * np.concatenate([[0], np.cumsum(KB)]).astype(np.int64)

    in_maps = []
    iota = np.tile(np.arange(P, dtype=np.float32), (P, 1)).astype(_bf16np)
    for c in range(NCORES):
        m = core == c
        tc_, rc_, gc_, hc_ = tt[m], rr[m], gidx[m], halfB[m]
        idx_flat = np.zeros(TA * P + TB * P, dtype=np.int16)
        dst_flat = np.full(TA * P + TB * P, -1.0, dtype=np.float32)
        for h, (K, base, off, goff) in enumerate(
                [(KA, baseA, 0, 0), (KB, baseB, TA * P, HALF)]):
            sel = hc_ == h
            th, rh, gh = tc_[sel], rc_[sel], gc_[sel] - goff
            o = np.argsort(th, kind="stable")
            th, rh, gh = th[o], rh[o], gh[o]
            per_tile = np.bincount(th, minlength=TPC)
            run_first = np.concatenate([[0], np.cumsum(per_tile)])[:-1]
            rank = np.arange(len(th)) - run_first[th]
            pos = off + base[th] + rank
            idx_flat[pos] = gh.astype(np.int16)
            dst_flat[pos] = rh.astype(np.float32)
        # wrap-16 index layout, replicated across the 8 gpsimd cores
        wrapped = idx_flat.reshape(-1, 16).T
        wrapped = np.tile(wrapped, (8, 1))
        dst_cols = dst_flat.reshape(TA + TB, P).T.astype(_bf16np)

        nloc = np.zeros(NPAD, dtype=np.float32)
        nloc[:NPC] = norm[c * NPC:(c + 1) * NPC]
        feat_s = np.zeros((NPAD, D), dtype=np.float32)
        feat_s[:NPC] = features[c * NPC:(c + 1) * NPC] * nloc[:NPC, None]
        in_maps.append({
            "feat": feat_s.astype(_bf16np),
            "idxd": np.ascontiguousarray(wrapped),
            "dstd": np.ascontiguousarray(dst_cols),
            "norm2d": np.ascontiguousarray(
                (nloc * nloc).reshape(TPC, P).T),
            "norm1d": np.ascontiguousarray(nloc.reshape(TPC, P).T),
            "iotad": iota,
            "w0": Ws[0].astype(_bf16np), "w1": Ws[1].astype(_bf16np),
            "w2": Ws[2].astype(_bf16np), "w3": Ws[3].astype(_bf16np),
        })
    return KA, KB, in_maps


class _Runner:
    """Caches the jitted shard_map executable for a compiled Bass module and
    provides one-shot runs (host in / host out) plus a chained-dispatch
    benchmark that amortizes the PJRT round-trip latency."""

    def __init__(self, nc):
        import jax
        from jax.sharding import Mesh, PartitionSpec, NamedSharding
        from jax.experimental.shard_map import shard_map
        from concourse.bass2jax import (
            _bass_exec_p, install_neuronx_cc_hook, partition_id_tensor)
        install_neuronx_cc_hook()
        self.jax = jax
        self.nc = nc
        partition_name = (nc.partition_id_tensor.name
                          if nc.partition_id_tensor else None)
        in_names, out_names, out_avals, zero_outs = [], [], [], []
        for alloc in nc.m.functions[0].allocations:
            if not isinstance(alloc, mybir.MemoryLocationSet):
                continue
            name = alloc.memorylocations[0].name
            if alloc.kind == "ExternalInput":
                if name != partition_name:
                    in_names.append(name)
            elif alloc.kind == "ExternalOutput":
                out_names.append(name)
                shape = tuple(alloc.tensor_shape)
                dtype = mybir.dt.np(alloc.dtype)
                out_avals.append(jax.core.ShapedArray(shape, dtype))
                zero_outs.append(np.zeros(shape, dtype))
        n_params = len(in_names)
        n_outs = len(out_avals)
        all_names = list(in_names) + list(out_names)
        if partition_name is not None:
            all_names.append(partition_name)
        donate = tuple(range(n_params, n_params + n_outs))
        self.in_names = in_names
        self.out_names = out_names
        self.out_avals = out_avals
        self.zero_outs = zero_outs
        self.n_params = n_params

        def _body(*args):
            operands = list(args)
            if partition_name is not None:
                operands.append(partition_id_tensor())
            outs = _bass_exec_p.bind(
                *operands, out_avals=tuple(out_avals),
                in_names=tuple(all_names), out_names=tuple(out_names),
                lowering_input_output_aliases=(),
                sim_require_finite=True, sim_require_nnan=True, nc=nc)
            return tuple(outs)

        devices = jax.devices()[:NCORES]
        mesh = Mesh(np.asarray(devices), ("core",))
        self.sharding = NamedSharding(mesh, PartitionSpec("core"))
        self.sharded = jax.jit(
            shard_map(_body, mesh=mesh,
                      in_specs=(PartitionSpec("core"),) * (n_params + n_outs),
                      out_specs=(PartitionSpec("core"),) * n_outs,
                      check_rep=False),
            donate_argnums=donate, keep_unused=True)
        self._staged = None

    def _concat_inputs(self, in_maps):
        return [np.concatenate([np.asarray(in_maps[c][n])
                                for c in range(NCORES)], axis=0)
                for n in self.in_names]

    def _fresh_zeros(self):
        return [np.zeros((NCORES * z.shape[0], *z.shape[1:]), z.dtype)
                for z in self.zero_outs]

    def run(self, in_maps):
        out_arrs = self.sharded(*self._concat_inputs(in_maps),
                                *self._fresh_zeros())
        return [
            {n: np.asarray(out_arrs[i]).reshape(
                NCORES, *self.out_avals[i].shape)[c]
             for i, n in enumerate(self.out_names)}
            for c in range(NCORES)]

    def stage(self, in_maps):
        jax = self.jax
        dev_in = [jax.device_put(x, self.sharding)
                  for x in self._concat_inputs(in_maps)]
        jax.block_until_ready(dev_in)
        self._staged = dev_in
        return dev_in

    def bench(self, in_maps, iters=50):
        """Best per-run wall time over chained executions: inputs staged on
        device, each run's output buffer donated to the next run (serialized
        on device by the data dependency), one sync at the end."""
        import time
        jax = self.jax
        dev_in = self.stage(in_maps)
        outs = [jax.device_put(z, self.sharding) for z in self._fresh_zeros()]
        jax.block_until_ready(outs)
        outs = self.sharded(*dev_in, *outs)
        jax.block_until_ready(outs)
        best = float("inf")
        for _ in range(3):
            t0 = time.time()
            cur = outs
            for _ in range(iters):
                cur = self.sharded(*dev_in, *cur)
            jax.block_until_ready(cur)
            best = min(best, (time.time() - t0) / iters)
            outs = cur
        return best


def kernel(features, edge_index, W0, W1, W2, W3):
    global _last_in_maps
    Ws = [np.ascontiguousarray(np.asarray(w, dtype=np.float32))
          for w in (W0, W1, W2, W3)]
    KA, KB, in_maps = _preprocess(features, edge_index, Ws)
    key = (tuple(KA), tuple(KB))
    if key not in _cache:
        _cache[key] = _build(KA, KB)
    nc = _cache[key]
    if key not in _runner_cache:
        _runner_cache[key] = _Runner(nc)
    runner = _runner_cache[key]
    _last_in_maps = in_maps
    res = runner.run(in_maps)
    out = np.concatenate(
        [res[c]["out"][:NPC].astype(np.float32) for c in range(NCORES)],
        axis=0)
    return out


def bench_ns(iters=50):
    """Benchmark the last-run configuration; returns best per-run ns."""
    assert _runner_cache and _last_in_maps is not None
    runner = next(iter(_runner_cache.values()))
    return int(runner.bench(_last_in_maps, iters=iters) * 1e9)


# revision 6
# speedup vs baseline: 352.3492x; 1.0718x over previous
import sys
sys.path.insert(0, "/opt/trn_rl_repo")
import numpy as np
import ml_dtypes

_bf16np = ml_dtypes.bfloat16

from contextlib import ExitStack
import concourse.tile as tile
from concourse import bass, bacc, mybir
from concourse.library_config import mlp

N = 50000
P = 128
NCORES = 8
NPC = N // NCORES            # 6250 nodes per core
TPC = (NPC + P - 1) // P     # 49 node tiles per core
NPAD = TPC * P               # 6272 padded nodes per core
NHL = NPAD // 2              # 3136: local node half
NFULL = NCORES * NPAD        # 50176 padded gather-source rows
HALF = NFULL // 2            # 25088 rows per gather table (< 2**15: int16 ok)
D = 128
DOUT = 40
CHUNK = 7                    # dst tiles per gather chunk
GMAX = 8                     # max slots (x128 idxs) per dma_gather

_cache = {}
_runner_cache = {}
_last_in_maps = None


def _build(KA, KB):
    """KA/KB: per-dst-tile slot counts (128 edges each) for gather half A
    (src in its core's first NHL nodes) and B (second half); shared across
    cores so the SPMD program is identical everywhere."""
    nc = bacc.Bacc("TRN2", target_bir_lowering=False, debug=False,
                   num_devices=NCORES, dynamic_dma_scratch_size=32768)
    f32, i16, bf16 = mybir.dt.float32, mybir.dt.int16, mybir.dt.bfloat16

    TA, TB = sum(KA), sum(KB)
    featrep = nc.dram_tensor("featrep", [NFULL, D], bf16,
                             kind="ExternalInput").ap()
    idxd = nc.dram_tensor("idxd", [P, (TA + TB) * 8], i16,
                          kind="ExternalInput").ap()
    dstd = nc.dram_tensor("dstd", [P, TA + TB], bf16,
                          kind="ExternalInput").ap()
    norm2d = nc.dram_tensor("norm2d", [P, TPC], f32, kind="ExternalInput").ap()
    norm1d = nc.dram_tensor("norm1d", [P, TPC], f32, kind="ExternalInput").ap()
    iotad = nc.dram_tensor("iotad", [P, P], bf16, kind="ExternalInput").ap()
    wd = [nc.dram_tensor(f"w{i}", [D, D if i < 3 else DOUT], bf16,
                         kind="ExternalInput").ap() for i in range(4)]
    outd = nc.dram_tensor("out", [NPAD, DOUT], bf16, kind="ExternalOutput").ap()

    # chunk boundaries over dst tiles, with per-chunk slot-column offsets
    chunks = []
    offA = offB = 0
    for c0 in range(0, TPC, CHUNK):
        ts = list(range(c0, min(c0 + CHUNK, TPC)))
        ka = sum(KA[t] for t in ts)
        kb = sum(KB[t] for t in ts)
        chunks.append((ts, offA, ka, offB, kb))
        offA += ka
        offB += kb

    with tile.TileContext(nc) as tc, ExitStack() as ctx:
        nc.gpsimd.load_library(mlp)
        dram = ctx.enter_context(tc.tile_pool(name="dram", bufs=2,
                                              space="DRAM"))
        shp = ctx.enter_context(tc.tile_pool(name="shp", bufs=2,
                                             space="DRAM"))
        consts = ctx.enter_context(tc.tile_pool(name="consts", bufs=1))
        hpool = ctx.enter_context(tc.tile_pool(name="hs", bufs=2))
        apool = ctx.enter_context(tc.tile_pool(name="aggA", bufs=2))
        msgpA = ctx.enter_context(tc.tile_pool(name="msgA", bufs=2))
        msgpB = ctx.enter_context(tc.tile_pool(name="msgB", bufs=2))
        selpA = ctx.enter_context(tc.tile_pool(name="selA", bufs=2))
        selpB = ctx.enter_context(tc.tile_pool(name="selB", bufs=2))
        aggp = ctx.enter_context(tc.tile_pool(name="agg", bufs=4))
        outp = ctx.enter_context(tc.tile_pool(name="outp", bufs=1))
        ps1 = ctx.enter_context(tc.tile_pool(name="ps1", bufs=3, space="PSUM"))
        ps2 = ctx.enter_context(tc.tile_pool(name="ps2", bufs=3, space="PSUM"))

        idx_sb = consts.tile([P, (TA + TB) * 8], i16)
        nc.sync.dma_start(idx_sb[:], idxd[:])
        dst_sb = consts.tile([P, TA + TB], bf16)
        nc.sync.dma_start(dst_sb[:], dstd[:])
        norm2_sb = consts.tile([P, TPC], f32)
        nc.sync.dma_start(norm2_sb[:], norm2d[:])
        norm1_sb = consts.tile([P, TPC], f32)
        nc.sync.dma_start(norm1_sb[:], norm1d[:])
        iota_sb = consts.tile([P, P], bf16)
        nc.sync.dma_start(iota_sb[:], iotad[:])
        w_sb = []
        for i in range(4):
            dcol = D if i < 3 else DOUT
            w = consts.tile([P, dcol], bf16)
            nc.sync.dma_start(w[:], wd[i][:])
            w_sb.append(w)
        out_sb = outp.tile([P, TPC * DOUT], bf16)

        def gather_and_sel(src_tab, K, off, kt, idx_base, msgp, selp):
            """One chunk-half: gather kt slots of 128 messages + build the
            one-hot dst selector in a single 3D-broadcast is_equal."""
            msg = msgp.tile([P, kt, D], bf16, name=f"msg{idx_base}")
            for s0 in range(0, kt, GMAX):
                sw = min(GMAX, kt - s0)
                nc.gpsimd.dma_gather(
                    msg[:, s0:s0 + sw, :], src_tab,
                    idx_sb[:, (idx_base + off + s0) * 8:
                           (idx_base + off + s0 + sw) * 8],
                    sw * P, sw * P, D)
            sel = selp.tile([P, kt * P], bf16, name=f"sel{idx_base}")
            nc.vector.tensor_tensor(
                out=sel[:].rearrange("p (n r) -> p n r", n=kt),
                in0=dst_sb[:, idx_base + off:idx_base + off + kt]
                    .unsqueeze(2).broadcast_to([P, kt, P]),
                in1=iota_sb[:].unsqueeze(1).broadcast_to([P, kt, P]),
                op=mybir.AluOpType.is_equal)
            return msg, sel

        def finish_tile(layer, t, aggT):
            dcol = D if layer < 3 else DOUT
            psO = ps2.tile([P, dcol], f32, space="PSUM")
            nc.tensor.matmul(out=psO[:], lhsT=aggT,
                             rhs=w_sb[layer][:, :dcol], start=True, stop=True)
            if layer < 3:
                nc.scalar.activation(
                    out=h_next[:, t * D:(t + 1) * D], in_=psO[:],
                    func=mybir.ActivationFunctionType.Relu,
                    scale=norm2_sb[:, t:t + 1])
            else:
                nc.scalar.activation(
                    out=out_sb[:, t * DOUT:(t + 1) * DOUT], in_=psO[:],
                    func=mybir.ActivationFunctionType.Copy,
                    scale=norm1_sb[:, t:t + 1])

        h_prev = None
        for layer in range(4):
            if layer == 0:
                tabA = featrep[0:HALF, :]
                tabB = featrep[HALF:NFULL, :]
            else:
                hfirst = shp.tile([HALF, D], bf16, addr_space="Shared")
                hsecond = shp.tile([HALF, D], bf16, addr_space="Shared")
                nc.gpsimd.collective_compute(
                    "AllGather", mybir.AluOpType.bypass,
                    replica_groups=[list(range(NCORES))],
                    ins=[h_prev[0:NHL, :]], outs=[hfirst[:]])
                tabA = hfirst[:]
                tabB = hsecond[:]
            if layer < 3:
                h_next = hpool.tile([P, TPC * D], bf16)

            if layer == 0:
                # both tables ready at start: single fused phase
                for ts, oA, ka, oB, kb in chunks:
                    msgA, selA = gather_and_sel(tabA, KA, oA, ka, 0,
                                                msgpA, selpA)
                    msgB, selB = gather_and_sel(tabB, KB, oB, kb, TA,
                                                msgpB, selpB)
                    jA = jB = 0
                    for t in ts:
                        nslots = KA[t] + KB[t]
                        if nslots == 0:
                            nc.vector.memset(h_next[:, t * D:(t + 1) * D], 0.0)
                            continue
                        ps = ps1.tile([P, P], f32, space="PSUM")
                        k = 0
                        for _ in range(KA[t]):
                            nc.tensor.matmul(
                                out=ps[:], lhsT=msgA[:, jA, :],
                                rhs=selA[:, jA * P:(jA + 1) * P],
                                start=(k == 0), stop=(k == nslots - 1))
                            jA += 1
                            k += 1
                        for _ in range(KB[t]):
                            nc.tensor.matmul(
                                out=ps[:], lhsT=msgB[:, jB, :],
                                rhs=selB[:, jB * P:(jB + 1) * P],
                                start=(k == 0), stop=(k == nslots - 1))
                            jB += 1
                            k += 1
                        aggT = aggp.tile([P, P], bf16)
                        nc.vector.tensor_copy(aggT[:], ps[:])
                        finish_tile(layer, t, aggT[:])
            else:
                # phase A: gather/accumulate first-half sources; overlaps the
                # second AllGather issued right after.
                aggA = apool.tile([P, TPC * P], bf16)
                for ts, oA, ka, oB, kb in chunks:
                    if ka == 0:
                        continue
                    msgA, selA = gather_and_sel(tabA, KA, oA, ka, 0,
                                                msgpA, selpA)
                    jA = 0
                    for t in ts:
                        if KA[t] == 0:
                            continue
                        ps = ps1.tile([P, P], f32, space="PSUM")
                        for k in range(KA[t]):
                            nc.tensor.matmul(
                                out=ps[:], lhsT=msgA[:, jA, :],
                                rhs=selA[:, jA * P:(jA + 1) * P],
                                start=(k == 0), stop=(k == KA[t] - 1))
                            jA += 1
                        nc.vector.tensor_copy(
                            aggA[:, t * P:(t + 1) * P], ps[:])
                nc.gpsimd.collective_compute(
                    "AllGather", mybir.AluOpType.bypass,
                    replica_groups=[list(range(NCORES))],
                    ins=[h_prev[NHL:NPAD, :]], outs=[hsecond[:]])
                # phase B: second-half sources, then combine + linear
                for ts, oA, ka, oB, kb in chunks:
                    if kb:
                        msgB, selB = gather_and_sel(tabB, KB, oB, kb, TA,
                                                    msgpB, selpB)
                    jB = 0
                    for t in ts:
                        if KA[t] == 0 and KB[t] == 0:
                            if layer < 3:
                                nc.vector.memset(
                                    h_next[:, t * D:(t + 1) * D], 0.0)
                            else:
                                nc.vector.memset(
                                    out_sb[:, t * DOUT:(t + 1) * DOUT], 0.0)
                            continue
                        if KB[t] == 0:
                            finish_tile(layer, t, aggA[:, t * P:(t + 1) * P])
                            continue
                        ps = ps1.tile([P, P], f32, space="PSUM")
                        for k in range(KB[t]):
                            nc.tensor.matmul(
                                out=ps[:], lhsT=msgB[:, jB, :],
                                rhs=selB[:, jB * P:(jB + 1) * P],
                                start=(k == 0), stop=(k == KB[t] - 1))
                            jB += 1
                        aggT = aggp.tile([P, P], bf16)
                        if KA[t] == 0:
                            nc.vector.tensor_copy(aggT[:], ps[:])
                        else:
                            nc.vector.tensor_tensor(
                                out=aggT[:], in0=aggA[:, t * P:(t + 1) * P],
                                in1=ps[:], op=mybir.AluOpType.add)
                        finish_tile(layer, t, aggT[:])
            if layer < 3:
                bounce = dram.tile([NPAD, D], bf16)
                nc.sync.dma_start(
                    bounce[:].rearrange("(t p) f -> p t f", p=P),
                    h_next[:].rearrange("p (t f) -> p t f", t=TPC))
                h_prev = bounce
        nc.sync.dma_start(
            outd[:].rearrange("(t p) f -> p t f", p=P),
            out_sb[:].rearrange("p (t f) -> p t f", t=TPC))
    nc.compile()
    return nc


def _preprocess(features, edge_index, Ws):
    src = np.asarray(edge_index[0], dtype=np.int64)
    dst = np.asarray(edge_index[1], dtype=np.int64)
    features = np.asarray(features, dtype=np.float32)

    deg = np.bincount(dst, minlength=N).astype(np.float32)
    norm = 1.0 / np.sqrt(np.maximum(deg, 1.0))

    core = dst // NPC
    tt = (dst - core * NPC) >> 7
    rr = (dst - core * NPC) & 127
    sc = src // NPC
    sl = src - sc * NPC
    halfB = (sl >= NHL).astype(np.int64)
    gidx = sc * NHL + sl - halfB * NHL   # row within its half-table

    key = (core * 2 + halfB) * TPC + tt
    cnt = np.bincount(key, minlength=NCORES * 2 * TPC).reshape(NCORES, 2, TPC)
    KA = [int(x) for x in -(-cnt[:, 0, :].max(axis=0) // P)]
    KB = [int(x) for x in -(-cnt[:, 1, :].max(axis=0) // P)]
    TA, TB = sum(KA), sum(KB)
    baseA = P * np.concatenate([[0], np.cumsum(KA)]).astype(np.int64)
    baseB = P * np.concatenate([[0], np.cumsum(KB)]).astype(np.int64)

    # replicated layer-0 gather table: first local halves of all cores,
    # then second local halves (matches the AllGather layout for layers 1-3)
    feat_s = features * norm[:, None]
    featrep = np.zeros((NFULL, D), dtype=np.float32)
    for c in range(NCORES):
        featrep[c * NHL:c * NHL + NHL] = feat_s[c * NPC:c * NPC + NHL]
        n2 = NPC - NHL
        featrep[HALF + c * NHL:HALF + c * NHL + n2] = \
            feat_s[c * NPC + NHL:(c + 1) * NPC]
    featrep = featrep.astype(_bf16np)

    in_maps = []
    iota = np.tile(np.arange(P, dtype=np.float32), (P, 1)).astype(_bf16np)
    for c in range(NCORES):
        m = core == c
        tc_, rc_, gc_, hc_ = tt[m], rr[m], gidx[m], halfB[m]
        idx_flat = np.zeros(TA * P + TB * P, dtype=np.int16)
        dst_flat = np.full(TA * P + TB * P, -1.0, dtype=np.float32)
        for h, (K, base, off) in enumerate(
                [(KA, baseA, 0), (KB, baseB, TA * P)]):
            sel = hc_ == h
            th, rh, gh = tc_[sel], rc_[sel], gc_[sel]
            o = np.argsort(th, kind="stable")
            th, rh, gh = th[o], rh[o], gh[o]
            per_tile = np.bincount(th, minlength=TPC)
            run_first = np.concatenate([[0], np.cumsum(per_tile)])[:-1]
            rank = np.arange(len(th)) - run_first[th]
            pos = off + base[th] + rank
            idx_flat[pos] = gh.astype(np.int16)
            dst_flat[pos] = rh.astype(np.float32)
        # wrap-16 index layout, replicated across the 8 gpsimd cores
        wrapped = np.tile(idx_flat.reshape(-1, 16).T, (8, 1))
        dst_cols = dst_flat.reshape(TA + TB, P).T.astype(_bf16np)

        nloc = np.zeros(NPAD, dtype=np.float32)
        nloc[:NPC] = norm[c * NPC:(c + 1) * NPC]
        in_maps.append({
            "featrep": featrep,
            "idxd": np.ascontiguousarray(wrapped),
            "dstd": np.ascontiguousarray(dst_cols),
            "norm2d": np.ascontiguousarray(
                (nloc * nloc).reshape(TPC, P).T),
            "norm1d": np.ascontiguousarray(nloc.reshape(TPC, P).T),
            "iotad": iota,
            "w0": Ws[0].astype(_bf16np), "w1": Ws[1].astype(_bf16np),
            "w2": Ws[2].astype(_bf16np), "w3": Ws[3].astype(_bf16np),
        })
    return KA, KB, in_maps


class _Runner:
    """Caches the jitted shard_map executable for a compiled Bass module and
    provides one-shot runs (host in / host out) plus a chained-dispatch
    benchmark that amortizes the PJRT round-trip latency."""

    def __init__(self, nc):
        import jax
        from jax.sharding import Mesh, PartitionSpec, NamedSharding
        from jax.experimental.shard_map import shard_map
        from concourse.bass2jax import (
            _bass_exec_p, install_neuronx_cc_hook, partition_id_tensor)
        install_neuronx_cc_hook()
        self.jax = jax
        self.nc = nc
        partition_name = (nc.partition_id_tensor.name
                          if nc.partition_id_tensor else None)
        in_names, out_names, out_avals, zero_outs = [], [], [], []
        for alloc in nc.m.functions[0].allocations:
            if not isinstance(alloc, mybir.MemoryLocationSet):
                continue
            name = alloc.memorylocations[0].name
            if alloc.kind == "ExternalInput":
                if name != partition_name:
                    in_names.append(name)
            elif alloc.kind == "ExternalOutput":
                out_names.append(name)
                shape = tuple(alloc.tensor_shape)
                dtype = mybir.dt.np(alloc.dtype)
                out_avals.append(jax.core.ShapedArray(shape, dtype))
                zero_outs.append(np.zeros(shape, dtype))
        n_params = len(in_names)
        n_outs = len(out_avals)
        all_names = list(in_names) + list(out_names)
        if partition_name is not None:
            all_names.append(partition_name)
        donate = tuple(range(n_params, n_params + n_outs))
        self.in_names = in_names
        self.out_names = out_names
        self.out_avals = out_avals
        self.zero_outs = zero_outs
        self.n_params = n_params

        def _body(*args):
            operands = list(args)
            if partition_name is not None:
                operands.append(partition_id_tensor())
            outs = _bass_exec_p.bind(
                *operands, out_avals=tuple(out_avals),
                in_names=tuple(all_names), out_names=tuple(out_names),
                lowering_input_output_aliases=(),
                sim_require_finite=True, sim_require_nnan=True, nc=nc)
            return tuple(outs)

        devices = jax.devices()[:NCORES]
        mesh = Mesh(np.asarray(devices), ("core",))
        self.sharding = NamedSharding(mesh, PartitionSpec("core"))
        self.sharded = jax.jit(
            shard_map(_body, mesh=mesh,
                      in_specs=(PartitionSpec("core"),) * (n_params + n_outs),
                      out_specs=(PartitionSpec("core"),) * n_outs,
                      check_rep=False),
            donate_argnums=donate, keep_unused=True)
        self._staged = None

    def _concat_inputs(self, in_maps):
        return [np.concatenate([np.asarray(in_maps[c][n])
                                for c in range(NCORES)], axis=0)
                for n in self.in_names]

    def _fresh_zeros(self):
        return [np.zeros((NCORES * z.shape[0], *z.shape[1:]), z.dtype)
                for z in self.zero_outs]

    def run(self, in_maps):
        out_arrs = self.sharded(*self._concat_inputs(in_maps),
                                *self._fresh_zeros())
        return [
            {n: np.asarray(out_arrs[i]).reshape(
                NCORES, *self.out_avals[i].shape)[c]
             for i, n in enumerate(self.out_names)}
            for c in range(NCORES)]

    def stage(self, in_maps):
        jax = self.jax
        dev_in = [jax.device_put(x, self.sharding)
                  for x in self._concat_inputs(in_maps)]
        jax.block_until_ready(dev_in)
        self._staged = dev_in
        return dev_in

    def bench(self, in_maps, iters=50):
        """Best per-run wall time over chained executions: inputs staged on
        device, each run's output buffer donated to the next run (serialized
        on device by the data dependency), one sync at the end."""
        import time
        jax = self.jax
        dev_in = self.stage(in_maps)
        outs = [jax.device_put(z, self.sharding) for z in self._fresh_zeros()]
        jax.block_until_ready(outs)
        outs = self.sharded(*dev_in, *outs)
        jax.block_until_ready(outs)
        best = float("inf")
        for _ in range(3):
            t0 = time.time()
            cur = outs
            for _ in range(iters):
                cur = self.sharded(*dev_in, *cur)
            jax.block_until_ready(cur)
            best = min(best, (time.time() - t0) / iters)
            outs = cur
        return best


def kernel(features, edge_index, W0, W1, W2, W3):
    global _last_in_maps
    Ws = [np.ascontiguousarray(np.asarray(w, dtype=np.float32))
          for w in (W0, W1, W2, W3)]
    KA, KB, in_maps = _preprocess(features, edge_index, Ws)
    key = (tuple(KA), tuple(KB))
    if key not in _cache:
        _cache[key] = _build(KA, KB)
    nc = _cache[key]
    if key not in _runner_cache:
        _runner_cache[key] = _Runner(nc)
    runner = _runner_cache[key]
    _last_in_maps = in_maps
    res = runner.run(in_maps)
    out = np.concatenate(
        [res[c]["out"][:NPC].astype(np.float32) for c in range(NCORES)],
        axis=0)
    return out


def bench_ns(iters=50):
    """Benchmark the last-run configuration; returns best per-run ns."""
    assert _runner_cache and _last_in_maps is not None
    runner = next(iter(_runner_cache.values()))
    return int(runner.bench(_last_in_maps, iters=iters) * 1e9)


# revision 7
# speedup vs baseline: 556.8239x; 1.5803x over previous
import sys
sys.path.insert(0, "/opt/trn_rl_repo")
import numpy as np
import ml_dtypes

_bf16np = ml_dtypes.bfloat16

from contextlib import ExitStack
import concourse.tile as tile
from concourse import bass, bacc, mybir
from concourse.library_config import mlp

N = 50000
P = 128
NCORES = 8
NPC = N // NCORES            # 6250 nodes per core
TPC = (NPC + P - 1) // P     # 49 node tiles per core
NPAD = TPC * P               # 6272 padded nodes per core
NHL = NPAD // 2              # 3136: local node half
NFULL = NCORES * NPAD        # 50176 padded gather-source rows
HALF = NFULL // 2            # 25088 rows per gather table (< 2**15: int16 ok)
D = 128
DOUT = 40
CHUNK = 7                    # dst tiles per gather chunk
GMAX = 8                     # max slots (x128 idxs) per dma_gather

_cache = {}
_runner_cache = {}
_last_in_maps = None


def _build(KA, KB):
    """KA/KB: per-dst-tile slot counts (128 edges each) for gather half A
    (src in its core's first NHL nodes) and B (second half); shared across
    cores so the SPMD program is identical everywhere."""
    nc = bacc.Bacc("TRN2", target_bir_lowering=False, debug=False,
                   num_devices=NCORES, dynamic_dma_scratch_size=32768,
                   num_swdge_queues=4)
    f32, i16, bf16 = mybir.dt.float32, mybir.dt.int16, mybir.dt.bfloat16

    TA, TB = sum(KA), sum(KB)
    featrep = nc.dram_tensor("featrep", [NFULL, D], bf16,
                             kind="ExternalInput").ap()
    idxd = nc.dram_tensor("idxd", [P, (TA + TB) * 8], i16,
                          kind="ExternalInput").ap()
    dstd = nc.dram_tensor("dstd", [P, TA + TB], bf16,
                          kind="ExternalInput").ap()
    norm2d = nc.dram_tensor("norm2d", [P, TPC], f32, kind="ExternalInput").ap()
    norm1d = nc.dram_tensor("norm1d", [P, TPC], f32, kind="ExternalInput").ap()
    iotad = nc.dram_tensor("iotad", [P, P], bf16, kind="ExternalInput").ap()
    wd = [nc.dram_tensor(f"w{i}", [D, D if i < 3 else DOUT], bf16,
                         kind="ExternalInput").ap() for i in range(4)]
    outd = nc.dram_tensor("out", [NPAD, DOUT], bf16, kind="ExternalOutput").ap()

    # chunk boundaries over dst tiles, with per-chunk slot-column offsets
    chunks = []
    offA = offB = 0
    for c0 in range(0, TPC, CHUNK):
        ts = list(range(c0, min(c0 + CHUNK, TPC)))
        ka = sum(KA[t] for t in ts)
        kb = sum(KB[t] for t in ts)
        chunks.append((ts, offA, ka, offB, kb))
        offA += ka
        offB += kb

    with tile.TileContext(nc) as tc, ExitStack() as ctx:
        nc.gpsimd.load_library(mlp)
        dram = ctx.enter_context(tc.tile_pool(name="dram", bufs=2,
                                              space="DRAM"))
        shp = ctx.enter_context(tc.tile_pool(name="shp", bufs=2,
                                             space="DRAM"))
        consts = ctx.enter_context(tc.tile_pool(name="consts", bufs=1))
        hpool = ctx.enter_context(tc.tile_pool(name="hs", bufs=2))
        apool = ctx.enter_context(tc.tile_pool(name="aggA", bufs=2))
        msgpA = ctx.enter_context(tc.tile_pool(name="msgA", bufs=2))
        msgpB = ctx.enter_context(tc.tile_pool(name="msgB", bufs=2))
        selpA = ctx.enter_context(tc.tile_pool(name="selA", bufs=2))
        selpB = ctx.enter_context(tc.tile_pool(name="selB", bufs=2))
        aggp = ctx.enter_context(tc.tile_pool(name="agg", bufs=4))
        outp = ctx.enter_context(tc.tile_pool(name="outp", bufs=1))
        ps1 = ctx.enter_context(tc.tile_pool(name="ps1", bufs=3, space="PSUM"))
        ps2 = ctx.enter_context(tc.tile_pool(name="ps2", bufs=3, space="PSUM"))

        idx_sb = consts.tile([P, (TA + TB) * 8], i16)
        nc.sync.dma_start(idx_sb[:], idxd[:])
        dst_sb = consts.tile([P, TA + TB], bf16)
        nc.sync.dma_start(dst_sb[:], dstd[:])
        norm2_sb = consts.tile([P, TPC], f32)
        nc.sync.dma_start(norm2_sb[:], norm2d[:])
        norm1_sb = consts.tile([P, TPC], f32)
        nc.sync.dma_start(norm1_sb[:], norm1d[:])
        iota_sb = consts.tile([P, P], bf16)
        nc.sync.dma_start(iota_sb[:], iotad[:])
        w_sb = []
        for i in range(4):
            dcol = D if i < 3 else DOUT
            w = consts.tile([P, dcol], bf16)
            nc.sync.dma_start(w[:], wd[i][:])
            w_sb.append(w)
        out_sb = outp.tile([P, TPC * DOUT], bf16)

        qrr = [0]

        def gather_and_sel(src_tab, K, off, kt, idx_base, msgp, selp):
            """One chunk-half: gather kt slots of 128 messages + build the
            one-hot dst selector in a single 3D-broadcast is_equal."""
            msg = msgp.tile([P, kt, D], bf16, name=f"msg{idx_base}")
            for s0 in range(0, kt, GMAX):
                sw = min(GMAX, kt - s0)
                nc.gpsimd.dma_gather(
                    msg[:, s0:s0 + sw, :], src_tab,
                    idx_sb[:, (idx_base + off + s0) * 8:
                           (idx_base + off + s0 + sw) * 8],
                    sw * P, sw * P, D, queue_num=qrr[0] % 4)
                qrr[0] += 1
            sel = selp.tile([P, kt * P], bf16, name=f"sel{idx_base}")
            nc.vector.tensor_tensor(
                out=sel[:].rearrange("p (n r) -> p n r", n=kt),
                in0=dst_sb[:, idx_base + off:idx_base + off + kt]
                    .unsqueeze(2).broadcast_to([P, kt, P]),
                in1=iota_sb[:].unsqueeze(1).broadcast_to([P, kt, P]),
                op=mybir.AluOpType.is_equal)
            return msg, sel

        def finish_tile(layer, t, aggT):
            dcol = D if layer < 3 else DOUT
            psO = ps2.tile([P, dcol], f32, space="PSUM")
            nc.tensor.matmul(out=psO[:], lhsT=aggT,
                             rhs=w_sb[layer][:, :dcol], start=True, stop=True)
            if layer < 3:
                nc.scalar.activation(
                    out=h_next[:, t * D:(t + 1) * D], in_=psO[:],
                    func=mybir.ActivationFunctionType.Relu,
                    scale=norm2_sb[:, t:t + 1])
            else:
                nc.scalar.activation(
                    out=out_sb[:, t * DOUT:(t + 1) * DOUT], in_=psO[:],
                    func=mybir.ActivationFunctionType.Copy,
                    scale=norm1_sb[:, t:t + 1])

        h_prev = None
        for layer in range(4):
            if layer == 0:
                tabA = featrep[0:HALF, :]
                tabB = featrep[HALF:NFULL, :]
            else:
                hfirst = shp.tile([HALF, D], bf16, addr_space="Shared")
                hsecond = shp.tile([HALF, D], bf16, addr_space="Shared")
                nc.gpsimd.collective_compute(
                    "AllGather", mybir.AluOpType.bypass,
                    replica_groups=[list(range(NCORES))],
                    ins=[h_prev[0:NHL, :]], outs=[hfirst[:]])
                tabA = hfirst[:]
                tabB = hsecond[:]
            if layer < 3:
                h_next = hpool.tile([P, TPC * D], bf16)

            if layer == 0:
                # both tables ready at start: single fused phase
                for ts, oA, ka, oB, kb in chunks:
                    msgA, selA = gather_and_sel(tabA, KA, oA, ka, 0,
                                                msgpA, selpA)
                    msgB, selB = gather_and_sel(tabB, KB, oB, kb, TA,
                                                msgpB, selpB)
                    jA = jB = 0
                    for t in ts:
                        nslots = KA[t] + KB[t]
                        if nslots == 0:
                            nc.vector.memset(h_next[:, t * D:(t + 1) * D], 0.0)
                            continue
                        ps = ps1.tile([P, P], f32, space="PSUM")
                        k = 0
                        for _ in range(KA[t]):
                            nc.tensor.matmul(
                                out=ps[:], lhsT=msgA[:, jA, :],
                                rhs=selA[:, jA * P:(jA + 1) * P],
                                start=(k == 0), stop=(k == nslots - 1))
                            jA += 1
                            k += 1
                        for _ in range(KB[t]):
                            nc.tensor.matmul(
                                out=ps[:], lhsT=msgB[:, jB, :],
                                rhs=selB[:, jB * P:(jB + 1) * P],
                                start=(k == 0), stop=(k == nslots - 1))
                            jB += 1
                            k += 1
                        aggT = aggp.tile([P, P], bf16)
                        nc.vector.tensor_copy(aggT[:], ps[:])
                        finish_tile(layer, t, aggT[:])
            else:
                # phase A: gather/accumulate first-half sources; overlaps the
                # second AllGather issued right after.
                aggA = apool.tile([P, TPC * P], bf16)
                for ts, oA, ka, oB, kb in chunks:
                    if ka == 0:
                        continue
                    msgA, selA = gather_and_sel(tabA, KA, oA, ka, 0,
                                                msgpA, selpA)
                    jA = 0
                    for t in ts:
                        if KA[t] == 0:
                            continue
                        ps = ps1.tile([P, P], f32, space="PSUM")
                        for k in range(KA[t]):
                            nc.tensor.matmul(
                                out=ps[:], lhsT=msgA[:, jA, :],
                                rhs=selA[:, jA * P:(jA + 1) * P],
                                start=(k == 0), stop=(k == KA[t] - 1))
                            jA += 1
                        nc.vector.tensor_copy(
                            aggA[:, t * P:(t + 1) * P], ps[:])
                nc.gpsimd.collective_compute(
                    "AllGather", mybir.AluOpType.bypass,
                    replica_groups=[list(range(NCORES))],
                    ins=[h_prev[NHL:NPAD, :]], outs=[hsecond[:]])
                # phase B: second-half sources, then combine + linear
                for ts, oA, ka, oB, kb in chunks:
                    if kb:
                        msgB, selB = gather_and_sel(tabB, KB, oB, kb, TA,
                                                    msgpB, selpB)
                    jB = 0
                    for t in ts:
                        if KA[t] == 0 and KB[t] == 0:
                            if layer < 3:
                                nc.vector.memset(
                                    h_next[:, t * D:(t + 1) * D], 0.0)
                            else:
                                nc.vector.memset(
                                    out_sb[:, t * DOUT:(t + 1) * DOUT], 0.0)
                            continue
                        if KB[t] == 0:
                            finish_tile(layer, t, aggA[:, t * P:(t + 1) * P])
                            continue
                        ps = ps1.tile([P, P], f32, space="PSUM")
                        for k in range(KB[t]):
                            nc.tensor.matmul(
                                out=ps[:], lhsT=msgB[:, jB, :],
                                rhs=selB[:, jB * P:(jB + 1) * P],
                                start=(k == 0), stop=(k == KB[t] - 1))
                            jB += 1
                        aggT = aggp.tile([P, P], bf16)
                        if KA[t] == 0:
                            nc.vector.tensor_copy(aggT[:], ps[:])
                        else:
                            nc.vector.tensor_tensor(
                                out=aggT[:], in0=aggA[:, t * P:(t + 1) * P],
                                in1=ps[:], op=mybir.AluOpType.add)
                        finish_tile(layer, t, aggT[:])
            if layer < 3:
                bounce = dram.tile([NPAD, D], bf16)
                nc.sync.dma_start(
                    bounce[:].rearrange("(t p) f -> p t f", p=P),
                    h_next[:].rearrange("p (t f) -> p t f", t=TPC))
                h_prev = bounce
        nc.sync.dma_start(
            outd[:].rearrange("(t p) f -> p t f", p=P),
            out_sb[:].rearrange("p (t f) -> p t f", t=TPC))
    nc.compile()
    return nc


def _preprocess(features, edge_index, Ws):
    src = np.asarray(edge_index[0], dtype=np.int64)
    dst = np.asarray(edge_index[1], dtype=np.int64)
    features = np.asarray(features, dtype=np.float32)

    deg = np.bincount(dst, minlength=N).astype(np.float32)
    norm = 1.0 / np.sqrt(np.maximum(deg, 1.0))

    core = dst // NPC
    tt = (dst - core * NPC) >> 7
    rr = (dst - core * NPC) & 127
    sc = src // NPC
    sl = src - sc * NPC
    halfB = (sl >= NHL).astype(np.int64)
    gidx = sc * NHL + sl - halfB * NHL   # row within its half-table

    key = (core * 2 + halfB) * TPC + tt
    cnt = np.bincount(key, minlength=NCORES * 2 * TPC).reshape(NCORES, 2, TPC)
    KA = [int(x) for x in -(-cnt[:, 0, :].max(axis=0) // P)]
    KB = [int(x) for x in -(-cnt[:, 1, :].max(axis=0) // P)]
    TA, TB = sum(KA), sum(KB)
    baseA = P * np.concatenate([[0], np.cumsum(KA)]).astype(np.int64)
    baseB = P * np.concatenate([[0], np.cumsum(KB)]).astype(np.int64)

    # replicated layer-0 gather table: first local halves of all cores,
    # then second local halves (matches the AllGather layout for layers 1-3)
    feat_s = features * norm[:, None]
    featrep = np.zeros((NFULL, D), dtype=np.float32)
    for c in range(NCORES):
        featrep[c * NHL:c * NHL + NHL] = feat_s[c * NPC:c * NPC + NHL]
        n2 = NPC - NHL
        featrep[HALF + c * NHL:HALF + c * NHL + n2] = \
            feat_s[c * NPC + NHL:(c + 1) * NPC]
    featrep = featrep.astype(_bf16np)

    in_maps = []
    iota = np.tile(np.arange(P, dtype=np.float32), (P, 1)).astype(_bf16np)
    for c in range(NCORES):
        m = core == c
        tc_, rc_, gc_, hc_ = tt[m], rr[m], gidx[m], halfB[m]
        idx_flat = np.zeros(TA * P + TB * P, dtype=np.int16)
        dst_flat = np.full(TA * P + TB * P, -1.0, dtype=np.float32)
        for h, (K, base, off) in enumerate(
                [(KA, baseA, 0), (KB, baseB, TA * P)]):
            sel = hc_ == h
            th, rh, gh = tc_[sel], rc_[sel], gc_[sel]
            o = np.lexsort((gh, th))
            th, rh, gh = th[o], rh[o], gh[o]
            per_tile = np.bincount(th, minlength=TPC)
            run_first = np.concatenate([[0], np.cumsum(per_tile)])[:-1]
            rank = np.arange(len(th)) - run_first[th]
            pos = off + base[th] + rank
            idx_flat[pos] = gh.astype(np.int16)
            dst_flat[pos] = rh.astype(np.float32)
        # wrap-16 index layout, replicated across the 8 gpsimd cores
        wrapped = np.tile(idx_flat.reshape(-1, 16).T, (8, 1))
        dst_cols = dst_flat.reshape(TA + TB, P).T.astype(_bf16np)

        nloc = np.zeros(NPAD, dtype=np.float32)
        nloc[:NPC] = norm[c * NPC:(c + 1) * NPC]
        in_maps.append({
            "featrep": featrep,
            "idxd": np.ascontiguousarray(wrapped),
            "dstd": np.ascontiguousarray(dst_cols),
            "norm2d": np.ascontiguousarray(
                (nloc * nloc).reshape(TPC, P).T),
            "norm1d": np.ascontiguousarray(nloc.reshape(TPC, P).T),
            "iotad": iota,
            "w0": Ws[0].astype(_bf16np), "w1": Ws[1].astype(_bf16np),
            "w2": Ws[2].astype(_bf16np), "w3": Ws[3].astype(_bf16np),
        })
    return KA, KB, in_maps


class _Runner:
    """Caches the jitted shard_map executable for a compiled Bass module and
    provides one-shot runs (host in / host out) plus a chained-dispatch
    benchmark that amortizes the PJRT round-trip latency."""

    def __init__(self, nc):
        import jax
        from jax.sharding import Mesh, PartitionSpec, NamedSharding
        from jax.experimental.shard_map import shard_map
        from concourse.bass2jax import (
            _bass_exec_p, install_neuronx_cc_hook, partition_id_tensor)
        install_neuronx_cc_hook()
        self.jax = jax
        self.nc = nc
        partition_name = (nc.partition_id_tensor.name
                          if nc.partition_id_tensor else None)
        in_names, out_names, out_avals, zero_outs = [], [], [], []
        for alloc in nc.m.functions[0].allocations:
            if not isinstance(alloc, mybir.MemoryLocationSet):
                continue
            name = alloc.memorylocations[0].name
            if alloc.kind == "ExternalInput":
                if name != partition_name:
                    in_names.append(name)
            elif alloc.kind == "ExternalOutput":
                out_names.append(name)
                shape = tuple(alloc.tensor_shape)
                dtype = mybir.dt.np(alloc.dtype)
                out_avals.append(jax.core.ShapedArray(shape, dtype))
                zero_outs.append(np.zeros(shape, dtype))
        n_params = len(in_names)
        n_outs = len(out_avals)
        all_names = list(in_names) + list(out_names)
        if partition_name is not None:
            all_names.append(partition_name)
        donate = tuple(range(n_params, n_params + n_outs))
        self.in_names = in_names
        self.out_names = out_names
        self.out_avals = out_avals
        self.zero_outs = zero_outs
        self.n_params = n_params

        def _body(*args):
            operands = list(args)
            if partition_name is not None:
                operands.append(partition_id_tensor())
            outs = _bass_exec_p.bind(
                *operands, out_avals=tuple(out_avals),
                in_names=tuple(all_names), out_names=tuple(out_names),
                lowering_input_output_aliases=(),
                sim_require_finite=True, sim_require_nnan=True, nc=nc)
            return tuple(outs)

        devices = jax.devices()[:NCORES]
        mesh = Mesh(np.asarray(devices), ("core",))
        self.sharding = NamedSharding(mesh, PartitionSpec("core"))
        self.sharded = jax.jit(
            shard_map(_body, mesh=mesh,
                      in_specs=(PartitionSpec("core"),) * (n_params + n_outs),
                      out_specs=(PartitionSpec("core"),) * n_outs,
                      check_rep=False),
            donate_argnums=donate, keep_unused=True)
        self._staged = None

    def _concat_inputs(self, in_maps):
        return [np.concatenate([np.asarray(in_maps[c][n])
                                for c in range(NCORES)], axis=0)
                for n in self.in_names]

    def _fresh_zeros(self):
        return [np.zeros((NCORES * z.shape[0], *z.shape[1:]), z.dtype)
                for z in self.zero_outs]

    def run(self, in_maps):
        out_arrs = self.sharded(*self._concat_inputs(in_maps),
                                *self._fresh_zeros())
        return [
            {n: np.asarray(out_arrs[i]).reshape(
                NCORES, *self.out_avals[i].shape)[c]
             for i, n in enumerate(self.out_names)}
            for c in range(NCORES)]

    def stage(self, in_maps):
        jax = self.jax
        dev_in = [jax.device_put(x, self.sharding)
                  for x in self._concat_inputs(in_maps)]
        jax.block_until_ready(dev_in)
        self._staged = dev_in
        return dev_in

    def bench(self, in_maps, iters=50):
        """Best per-run wall time over chained executions: inputs staged on
        device, each run's output buffer donated to the next run (serialized
        on device by the data dependency), one sync at the end."""
        import time
        jax = self.jax
        dev_in = self.stage(in_maps)
        outs = [jax.device_put(z, self.sharding) for z in self._fresh_zeros()]
        jax.block_until_ready(outs)
        outs = self.sharded(*dev_in, *outs)
        jax.block_until_ready(outs)
        best = float("inf")
        for _ in range(3):
            t0 = time.time()
            cur = outs
            for _ in range(iters):
                cur = self.sharded(*dev_in, *cur)
            jax.block_until_ready(cur)
            best = min(best, (time.time() - t0) / iters)
            outs = cur
        return best


def kernel(features, edge_index, W0, W1, W2, W3):
    global _last_in_maps
    Ws = [np.ascontiguousarray(np.asarray(w, dtype=np.float32))
          for w in (W0, W1, W2, W3)]
    KA, KB, in_maps = _preprocess(features, edge_index, Ws)
    key = (tuple(KA), tuple(KB))
    if key not in _cache:
        _cache[key] = _build(KA, KB)
    nc = _cache[key]
    if key not in _runner_cache:
        _runner_cache[key] = _Runner(nc)
    runner = _runner_cache[key]
    _last_in_maps = in_maps
    res = runner.run(in_maps)
    out = np.concatenate(
        [res[c]["out"][:NPC].astype(np.float32) for c in range(NCORES)],
        axis=0)
    return out


def bench_ns(iters=50):
    """Benchmark the last-run configuration; returns best per-run ns."""
    assert _runner_cache and _last_in_maps is not None
    runner = next(iter(_runner_cache.values()))
    return int(runner.bench(_last_in_maps, iters=iters) * 1e9)


# revision 8
# speedup vs baseline: 587.5250x; 1.0551x over previous
import sys
sys.path.insert(0, "/opt/trn_rl_repo")
import numpy as np
import ml_dtypes

_bf16np = ml_dtypes.bfloat16

from contextlib import ExitStack
import concourse.tile as tile
from concourse import bass, bacc, mybir
from concourse.library_config import mlp

N = 50000
P = 128
NCORES = 8
NPC = N // NCORES            # 6250 nodes per core
TPC = (NPC + P - 1) // P     # 49 node tiles per core
NPAD = TPC * P               # 6272 padded nodes per core
NHL = NPAD // 2              # 3136: local node half
NFULL = NCORES * NPAD        # 50176 padded gather-source rows
HALF = NFULL // 2            # 25088 rows per gather table (< 2**15: int16 ok)
D = 128
DOUT = 40
CHUNK = 7                    # dst tiles per gather chunk
GMAX = 8                     # max slots (x128 idxs) per dma_gather

_cache = {}
_runner_cache = {}
_last_in_maps = None


def _build(KA, KB):
    """KA/KB: per-dst-tile slot counts (128 edges each) for gather half A
    (src in its core's first NHL nodes) and B (second half); shared across
    cores so the SPMD program is identical everywhere."""
    nc = bacc.Bacc("TRN2", target_bir_lowering=False, debug=False,
                   num_devices=NCORES, dynamic_dma_scratch_size=32768,
                   num_swdge_queues=4)
    f32, i16, bf16 = mybir.dt.float32, mybir.dt.int16, mybir.dt.bfloat16

    TA, TB = sum(KA), sum(KB)
    featrep = nc.dram_tensor("featrep", [NFULL, D], bf16,
                             kind="ExternalInput").ap()
    idxd = nc.dram_tensor("idxd", [P, (TA + TB) * 8], i16,
                          kind="ExternalInput").ap()
    dstd = nc.dram_tensor("dstd", [P, TA + TB], bf16,
                          kind="ExternalInput").ap()
    norm2d = nc.dram_tensor("norm2d", [P, TPC], f32, kind="ExternalInput").ap()
    norm1d = nc.dram_tensor("norm1d", [P, TPC], f32, kind="ExternalInput").ap()
    iotad = nc.dram_tensor("iotad", [P, P], bf16, kind="ExternalInput").ap()
    wd = [nc.dram_tensor(f"w{i}", [D, D if i < 3 else DOUT], bf16,
                         kind="ExternalInput").ap() for i in range(4)]
    outd = nc.dram_tensor("out", [NPAD, DOUT], bf16, kind="ExternalOutput").ap()

    # chunk boundaries over dst tiles, with per-chunk slot-column offsets
    chunks = []
    offA = offB = 0
    for c0 in range(0, TPC, CHUNK):
        ts = list(range(c0, min(c0 + CHUNK, TPC)))
        ka = sum(KA[t] for t in ts)
        kb = sum(KB[t] for t in ts)
        chunks.append((ts, offA, ka, offB, kb))
        offA += ka
        offB += kb

    with tile.TileContext(nc) as tc, ExitStack() as ctx:
        nc.gpsimd.load_library(mlp)
        dram = ctx.enter_context(tc.tile_pool(name="dram", bufs=2,
                                              space="DRAM"))
        shp = ctx.enter_context(tc.tile_pool(name="shp", bufs=2,
                                             space="DRAM"))
        consts = ctx.enter_context(tc.tile_pool(name="consts", bufs=1))
        hpool = ctx.enter_context(tc.tile_pool(name="hs", bufs=2))
        apool = ctx.enter_context(tc.tile_pool(name="aggA", bufs=2))
        msgpA = ctx.enter_context(tc.tile_pool(name="msgA", bufs=2))
        msgpB = ctx.enter_context(tc.tile_pool(name="msgB", bufs=2))
        selpA = ctx.enter_context(tc.tile_pool(name="selA", bufs=2))
        selpB = ctx.enter_context(tc.tile_pool(name="selB", bufs=2))
        aggp = ctx.enter_context(tc.tile_pool(name="agg", bufs=4))
        outp = ctx.enter_context(tc.tile_pool(name="outp", bufs=1))
        ps1 = ctx.enter_context(tc.tile_pool(name="ps1", bufs=3, space="PSUM"))
        ps2 = ctx.enter_context(tc.tile_pool(name="ps2", bufs=3, space="PSUM"))

        idx_sb = consts.tile([P, (TA + TB) * 8], i16)
        nc.sync.dma_start(idx_sb[:], idxd[:])
        dst_sb = consts.tile([P, TA + TB], bf16)
        nc.sync.dma_start(dst_sb[:], dstd[:])
        norm2_sb = consts.tile([P, TPC], f32)
        nc.sync.dma_start(norm2_sb[:], norm2d[:])
        norm1_sb = consts.tile([P, TPC], f32)
        nc.sync.dma_start(norm1_sb[:], norm1d[:])
        iota_sb = consts.tile([P, P], bf16)
        nc.sync.dma_start(iota_sb[:], iotad[:])
        w_sb = []
        for i in range(4):
            dcol = D if i < 3 else DOUT
            w = consts.tile([P, dcol], bf16)
            nc.sync.dma_start(w[:], wd[i][:])
            w_sb.append(w)
        out_sb = outp.tile([P, TPC * DOUT], bf16)

        qrr = [0]

        def gather_and_sel(src_tab, K, off, kt, idx_base, msgp, selp):
            """One chunk-half: gather kt slots of 128 messages + build the
            one-hot dst selector in a single 3D-broadcast is_equal."""
            msg = msgp.tile([P, kt, D], bf16, name=f"msg{idx_base}")
            for s0 in range(0, kt, GMAX):
                sw = min(GMAX, kt - s0)
                nc.gpsimd.dma_gather(
                    msg[:, s0:s0 + sw, :], src_tab,
                    idx_sb[:, (idx_base + off + s0) * 8:
                           (idx_base + off + s0 + sw) * 8],
                    sw * P, sw * P, D, queue_num=qrr[0] % 4,
                    single_packet=False)
                qrr[0] += 1
            sel = selp.tile([P, kt * P], bf16, name=f"sel{idx_base}")
            nc.vector.tensor_tensor(
                out=sel[:].rearrange("p (n r) -> p n r", n=kt),
                in0=dst_sb[:, idx_base + off:idx_base + off + kt]
                    .unsqueeze(2).broadcast_to([P, kt, P]),
                in1=iota_sb[:].unsqueeze(1).broadcast_to([P, kt, P]),
                op=mybir.AluOpType.is_equal)
            return msg, sel

        def finish_tile(layer, t, aggT):
            dcol = D if layer < 3 else DOUT
            psO = ps2.tile([P, dcol], f32, space="PSUM")
            nc.tensor.matmul(out=psO[:], lhsT=aggT,
                             rhs=w_sb[layer][:, :dcol], start=True, stop=True)
            if layer < 3:
                nc.scalar.activation(
                    out=h_next[:, t * D:(t + 1) * D], in_=psO[:],
                    func=mybir.ActivationFunctionType.Relu,
                    scale=norm2_sb[:, t:t + 1])
            else:
                nc.scalar.activation(
                    out=out_sb[:, t * DOUT:(t + 1) * DOUT], in_=psO[:],
                    func=mybir.ActivationFunctionType.Copy,
                    scale=norm1_sb[:, t:t + 1])

        h_prev = None
        for layer in range(4):
            if layer == 0:
                tabA = featrep[0:HALF, :]
                tabB = featrep[HALF:NFULL, :]
            else:
                hfirst = shp.tile([HALF, D], bf16, addr_space="Shared")
                hsecond = shp.tile([HALF, D], bf16, addr_space="Shared")
                nc.gpsimd.collective_compute(
                    "AllGather", mybir.AluOpType.bypass,
                    replica_groups=[list(range(NCORES))],
                    ins=[h_prev[0:NHL, :]], outs=[hfirst[:]])
                tabA = hfirst[:]
                tabB = hsecond[:]
            if layer < 3:
                h_next = hpool.tile([P, TPC * D], bf16)

            if layer == 0:
                # both tables ready at start: single fused phase
                for ts, oA, ka, oB, kb in chunks:
                    msgA, selA = gather_and_sel(tabA, KA, oA, ka, 0,
                                                msgpA, selpA)
                    msgB, selB = gather_and_sel(tabB, KB, oB, kb, TA,
                                                msgpB, selpB)
                    jA = jB = 0
                    for t in ts:
                        nslots = KA[t] + KB[t]
                        if nslots == 0:
                            nc.vector.memset(h_next[:, t * D:(t + 1) * D], 0.0)
                            continue
                        ps = ps1.tile([P, P], f32, space="PSUM")
                        k = 0
                        for _ in range(KA[t]):
                            nc.tensor.matmul(
                                out=ps[:], lhsT=msgA[:, jA, :],
                                rhs=selA[:, jA * P:(jA + 1) * P],
                                start=(k == 0), stop=(k == nslots - 1))
                            jA += 1
                            k += 1
                        for _ in range(KB[t]):
                            nc.tensor.matmul(
                                out=ps[:], lhsT=msgB[:, jB, :],
                                rhs=selB[:, jB * P:(jB + 1) * P],
                                start=(k == 0), stop=(k == nslots - 1))
                            jB += 1
                            k += 1
                        aggT = aggp.tile([P, P], bf16)
                        nc.vector.tensor_copy(aggT[:], ps[:])
                        finish_tile(layer, t, aggT[:])
            else:
                # phase A: gather/accumulate first-half sources; overlaps the
                # second AllGather issued right after.
                aggA = apool.tile([P, TPC * P], bf16)
                for ts, oA, ka, oB, kb in chunks:
                    if ka == 0:
                        continue
                    msgA, selA = gather_and_sel(tabA, KA, oA, ka, 0,
                                                msgpA, selpA)
                    jA = 0
                    for t in ts:
                        if KA[t] == 0:
                            continue
                        ps = ps1.tile([P, P], f32, space="PSUM")
                        for k in range(KA[t]):
                            nc.tensor.matmul(
                                out=ps[:], lhsT=msgA[:, jA, :],
                                rhs=selA[:, jA * P:(jA + 1) * P],
                                start=(k == 0), stop=(k == KA[t] - 1))
                            jA += 1
                        nc.vector.tensor_copy(
                            aggA[:, t * P:(t + 1) * P], ps[:])
                nc.gpsimd.collective_compute(
                    "AllGather", mybir.AluOpType.bypass,
                    replica_groups=[list(range(NCORES))],
                    ins=[h_prev[NHL:NPAD, :]], outs=[hsecond[:]])
                # phase B: second-half sources, then combine + linear
                for ts, oA, ka, oB, kb in chunks:
                    if kb:
                        msgB, selB = gather_and_sel(tabB, KB, oB, kb, TA,
                                                    msgpB, selpB)
                    jB = 0
                    for t in ts:
                        if KA[t] == 0 and KB[t] == 0:
                            if layer < 3:
                                nc.vector.memset(
                                    h_next[:, t * D:(t + 1) * D], 0.0)
                            else:
                                nc.vector.memset(
                                    out_sb[:, t * DOUT:(t + 1) * DOUT], 0.0)
                            continue
                        if KB[t] == 0:
                            finish_tile(layer, t, aggA[:, t * P:(t + 1) * P])
                            continue
                        ps = ps1.tile([P, P], f32, space="PSUM")
                        for k in range(KB[t]):
                            nc.tensor.matmul(
                                out=ps[:], lhsT=msgB[:, jB, :],
                                rhs=selB[:, jB * P:(jB + 1) * P],
                                start=(k == 0), stop=(k == KB[t] - 1))
                            jB += 1
                        aggT = aggp.tile([P, P], bf16)
                        if KA[t] == 0:
                            nc.vector.tensor_copy(aggT[:], ps[:])
                        else:
                            nc.vector.tensor_tensor(
                                out=aggT[:], in0=aggA[:, t * P:(t + 1) * P],
                                in1=ps[:], op=mybir.AluOpType.add)
                        finish_tile(layer, t, aggT[:])
            if layer < 3:
                bounce = dram.tile([NPAD, D], bf16)
                nc.sync.dma_start(
                    bounce[:].rearrange("(t p) f -> p t f", p=P),
                    h_next[:].rearrange("p (t f) -> p t f", t=TPC))
                h_prev = bounce
        nc.sync.dma_start(
            outd[:].rearrange("(t p) f -> p t f", p=P),
            out_sb[:].rearrange("p (t f) -> p t f", t=TPC))
    nc.compile()
    return nc


def _preprocess(features, edge_index, Ws):
    src = np.asarray(edge_index[0], dtype=np.int64)
    dst = np.asarray(edge_index[1], dtype=np.int64)
    features = np.asarray(features, dtype=np.float32)

    deg = np.bincount(dst, minlength=N).astype(np.float32)
    norm = 1.0 / np.sqrt(np.maximum(deg, 1.0))

    core = dst // NPC
    tt = (dst - core * NPC) >> 7
    rr = (dst - core * NPC) & 127
    sc = src // NPC
    sl = src - sc * NPC
    halfB = (sl >= NHL).astype(np.int64)
    gidx = sc * NHL + sl - halfB * NHL   # row within its half-table

    key = (core * 2 + halfB) * TPC + tt
    cnt = np.bincount(key, minlength=NCORES * 2 * TPC).reshape(NCORES, 2, TPC)
    KA = [int(x) for x in -(-cnt[:, 0, :].max(axis=0) // P)]
    KB = [int(x) for x in -(-cnt[:, 1, :].max(axis=0) // P)]
    TA, TB = sum(KA), sum(KB)
    baseA = P * np.concatenate([[0], np.cumsum(KA)]).astype(np.int64)
    baseB = P * np.concatenate([[0], np.cumsum(KB)]).astype(np.int64)

    # replicated layer-0 gather table: first local halves of all cores,
    # then second local halves (matches the AllGather layout for layers 1-3)
    feat_s = features * norm[:, None]
    featrep = np.zeros((NFULL, D), dtype=np.float32)
    for c in range(NCORES):
        featrep[c * NHL:c * NHL + NHL] = feat_s[c * NPC:c * NPC + NHL]
        n2 = NPC - NHL
        featrep[HALF + c * NHL:HALF + c * NHL + n2] = \
            feat_s[c * NPC + NHL:(c + 1) * NPC]
    featrep = featrep.astype(_bf16np)

    in_maps = []
    iota = np.tile(np.arange(P, dtype=np.float32), (P, 1)).astype(_bf16np)
    for c in range(NCORES):
        m = core == c
        tc_, rc_, gc_, hc_ = tt[m], rr[m], gidx[m], halfB[m]
        idx_flat = np.zeros(TA * P + TB * P, dtype=np.int16)
        dst_flat = np.full(TA * P + TB * P, -1.0, dtype=np.float32)
        for h, (K, base, off) in enumerate(
                [(KA, baseA, 0), (KB, baseB, TA * P)]):
            sel = hc_ == h
            th, rh, gh = tc_[sel], rc_[sel], gc_[sel]
            o = np.lexsort((gh, th))
            th, rh, gh = th[o], rh[o], gh[o]
            per_tile = np.bincount(th, minlength=TPC)
            run_first = np.concatenate([[0], np.cumsum(per_tile)])[:-1]
            rank = np.arange(len(th)) - run_first[th]
            pos = off + base[th] + rank
            idx_flat[pos] = gh.astype(np.int16)
            dst_flat[pos] = rh.astype(np.float32)
        # wrap-16 index layout, replicated across the 8 gpsimd cores
        wrapped = np.tile(idx_flat.reshape(-1, 16).T, (8, 1))
        dst_cols = dst_flat.reshape(TA + TB, P).T.astype(_bf16np)

        nloc = np.zeros(NPAD, dtype=np.float32)
        nloc[:NPC] = norm[c * NPC:(c + 1) * NPC]
        in_maps.append({
            "featrep": featrep,
            "idxd": np.ascontiguousarray(wrapped),
            "dstd": np.ascontiguousarray(dst_cols),
            "norm2d": np.ascontiguousarray(
                (nloc * nloc).reshape(TPC, P).T),
            "norm1d": np.ascontiguousarray(nloc.reshape(TPC, P).T),
            "iotad": iota,
            "w0": Ws[0].astype(_bf16np), "w1": Ws[1].astype(_bf16np),
            "w2": Ws[2].astype(_bf16np), "w3": Ws[3].astype(_bf16np),
        })
    return KA, KB, in_maps


class _Runner:
    """Caches the jitted shard_map executable for a compiled Bass module and
    provides one-shot runs (host in / host out) plus a chained-dispatch
    benchmark that amortizes the PJRT round-trip latency."""

    def __init__(self, nc):
        import jax
        from jax.sharding import Mesh, PartitionSpec, NamedSharding
        from jax.experimental.shard_map import shard_map
        from concourse.bass2jax import (
            _bass_exec_p, install_neuronx_cc_hook, partition_id_tensor)
        install_neuronx_cc_hook()
        self.jax = jax
        self.nc = nc
        partition_name = (nc.partition_id_tensor.name
                          if nc.partition_id_tensor else None)
        in_names, out_names, out_avals, zero_outs = [], [], [], []
        for alloc in nc.m.functions[0].allocations:
            if not isinstance(alloc, mybir.MemoryLocationSet):
                continue
            name = alloc.memorylocations[0].name
            if alloc.kind == "ExternalInput":
                if name != partition_name:
                    in_names.append(name)
            elif alloc.kind == "ExternalOutput":
                out_names.append(name)
                shape = tuple(alloc.tensor_shape)
                dtype = mybir.dt.np(alloc.dtype)
                out_avals.append(jax.core.ShapedArray(shape, dtype))
                zero_outs.append(np.zeros(shape, dtype))
        n_params = len(in_names)
        n_outs = len(out_avals)
        all_names = list(in_names) + list(out_names)
        if partition_name is not None:
            all_names.append(partition_name)
        donate = tuple(range(n_params, n_params + n_outs))
        self.in_names = in_names
        self.out_names = out_names
        self.out_avals = out_avals
        self.zero_outs = zero_outs
        self.n_params = n_params

        def _body(*args):
            operands = list(args)
            if partition_name is not None:
                operands.append(partition_id_tensor())
            outs = _bass_exec_p.bind(
                *operands, out_avals=tuple(out_avals),
                in_names=tuple(all_names), out_names=tuple(out_names),
                lowering_input_output_aliases=(),
                sim_require_finite=True, sim_require_nnan=True, nc=nc)
            return tuple(outs)

        devices = jax.devices()[:NCORES]
        mesh = Mesh(np.asarray(devices), ("core",))
        self.sharding = NamedSharding(mesh, PartitionSpec("core"))
        self.sharded = jax.jit(
            shard_map(_body, mesh=mesh,
                      in_specs=(PartitionSpec("core"),) * (n_params + n_outs),
                      out_specs=(PartitionSpec("core"),) * n_outs,
                      check_rep=False),
            donate_argnums=donate, keep_unused=True)
        self._staged = None

    def _concat_inputs(self, in_maps):
        return [np.concatenate([np.asarray(in_maps[c][n])
                                for c in range(NCORES)], axis=0)
                for n in self.in_names]

    def _fresh_zeros(self):
        return [np.zeros((NCORES * z.shape[0], *z.shape[1:]), z.dtype)
                for z in self.zero_outs]

    def run(self, in_maps):
        out_arrs = self.sharded(*self._concat_inputs(in_maps),
                                *self._fresh_zeros())
        return [
            {n: np.asarray(out_arrs[i]).reshape(
                NCORES, *self.out_avals[i].shape)[c]
             for i, n in enumerate(self.out_names)}
            for c in range(NCORES)]

    def stage(self, in_maps):
        jax = self.jax
        dev_in = [jax.device_put(x, self.sharding)
                  for x in self._concat_inputs(in_maps)]
        jax.block_until_ready(dev_in)
        self._staged = dev_in
        return dev_in

    def bench(self, in_maps, iters=50):
        """Best per-run wall time over chained executions: inputs staged on
        device, each run's output buffer donated to the next run (serialized
        on device by the data dependency), one sync at the end."""
        import time
        jax = self.jax
        dev_in = self.stage(in_maps)
        outs = [jax.device_put(z, self.sharding) for z in self._fresh_zeros()]
        jax.block_until_ready(outs)
        outs = self.sharded(*dev_in, *outs)
        jax.block_until_ready(outs)
        best = float("inf")
        for _ in range(3):
            t0 = time.time()
            cur = outs
            for _ in range(iters):
                cur = self.sharded(*dev_in, *cur)
            jax.block_until_ready(cur)
            best = min(best, (time.time() - t0) / iters)
            outs = cur
        return best


def kernel(features, edge_index, W0, W1, W2, W3):
    global _last_in_maps
    Ws = [np.ascontiguousarray(np.asarray(w, dtype=np.float32))
          for w in (W0, W1, W2, W3)]
    KA, KB, in_maps = _preprocess(features, edge_index, Ws)
    key = (tuple(KA), tuple(KB))
    if key not in _cache:
        _cache[key] = _build(KA, KB)
    nc = _cache[key]
    if key not in _runner_cache:
        _runner_cache[key] = _Runner(nc)
    runner = _runner_cache[key]
    _last_in_maps = in_maps
    res = runner.run(in_maps)
    out = np.concatenate(
        [res[c]["out"][:NPC].astype(np.float32) for c in range(NCORES)],
        axis=0)
    return out


def bench_ns(iters=50):
    """Benchmark the last-run configuration; returns best per-run ns."""
    assert _runner_cache and _last_in_maps is not None
    runner = next(iter(_runner_cache.values()))
    return int(runner.bench(_last_in_maps, iters=iters) * 1e9)


# revision 11
# speedup vs baseline: 635.8273x; 1.0822x over previous
import sys
sys.path.insert(0, "/opt/trn_rl_repo")
import numpy as np
import ml_dtypes

_bf16np = ml_dtypes.bfloat16

from contextlib import ExitStack
import concourse.tile as tile
from concourse import bass, bacc, mybir
from concourse.library_config import mlp

N = 50000
P = 128
NCORES = 8
NPC = N // NCORES            # 6250 nodes per core
TPC = (NPC + P - 1) // P     # 49 node tiles per core
NPAD = TPC * P               # 6272 padded nodes per core
NHL = NPAD // 2              # 3136: local node half
NFULL = NCORES * NPAD        # 50176 padded gather-source rows
HALF = NFULL // 2            # 25088 rows per gather table (< 2**15: int16 ok)
D = 128
DOUT = 40
CHUNK = 7                    # dst tiles per gather chunk
GMAX = 8                     # max slots (x128 idxs) per dma_gather

_cache = {}
_runner_cache = {}
_last_in_maps = None


def _build(KA, KB):
    """KA/KB: per-dst-tile slot counts (128 edges each) for gather half A
    (src in its core's first NHL nodes) and B (second half); shared across
    cores so the SPMD program is identical everywhere."""
    nc = bacc.Bacc("TRN2", target_bir_lowering=False, debug=False,
                   num_devices=NCORES, dynamic_dma_scratch_size=32768,
                   num_swdge_queues=4)
    f32, i16, bf16 = mybir.dt.float32, mybir.dt.int16, mybir.dt.bfloat16

    TA, TB = sum(KA), sum(KB)
    featrep = nc.dram_tensor("featrep", [NFULL, D], bf16,
                             kind="ExternalInput").ap()
    idxd = nc.dram_tensor("idxd", [P, (TA + TB) * 8], i16,
                          kind="ExternalInput").ap()
    dstd = nc.dram_tensor("dstd", [P, TA + TB], bf16,
                          kind="ExternalInput").ap()
    norm2d = nc.dram_tensor("norm2d", [P, TPC], f32, kind="ExternalInput").ap()
    norm1d = nc.dram_tensor("norm1d", [P, TPC], f32, kind="ExternalInput").ap()
    iotad = nc.dram_tensor("iotad", [P, P], bf16, kind="ExternalInput").ap()
    wd = [nc.dram_tensor(f"w{i}", [D, D if i < 3 else DOUT], bf16,
                         kind="ExternalInput").ap() for i in range(4)]
    outd = nc.dram_tensor("out", [NPAD, DOUT], bf16, kind="ExternalOutput").ap()

    # chunk boundaries over dst tiles, with per-chunk slot-column offsets
    chunks = []
    offA = offB = 0
    for c0 in range(0, TPC, CHUNK):
        ts = list(range(c0, min(c0 + CHUNK, TPC)))
        ka = sum(KA[t] for t in ts)
        kb = sum(KB[t] for t in ts)
        chunks.append((ts, offA, ka, offB, kb))
        offA += ka
        offB += kb

    with tile.TileContext(nc) as tc, ExitStack() as ctx:
        nc.gpsimd.load_library(mlp)
        dram = ctx.enter_context(tc.tile_pool(name="dram", bufs=2,
                                              space="DRAM"))
        shp = ctx.enter_context(tc.tile_pool(name="shp", bufs=2,
                                             space="DRAM"))
        consts = ctx.enter_context(tc.tile_pool(name="consts", bufs=1))
        hpool = ctx.enter_context(tc.tile_pool(name="hs", bufs=2))
        apool = ctx.enter_context(tc.tile_pool(name="aggA", bufs=1))
        msgpA = ctx.enter_context(tc.tile_pool(name="msgA", bufs=3))
        msgpB = ctx.enter_context(tc.tile_pool(name="msgB", bufs=3))
        selpA = ctx.enter_context(tc.tile_pool(name="selA", bufs=2))
        selpB = ctx.enter_context(tc.tile_pool(name="selB", bufs=2))
        aggp = ctx.enter_context(tc.tile_pool(name="agg", bufs=4))
        outp = ctx.enter_context(tc.tile_pool(name="outp", bufs=1))
        ps1 = ctx.enter_context(tc.tile_pool(name="ps1", bufs=3, space="PSUM"))
        ps2 = ctx.enter_context(tc.tile_pool(name="ps2", bufs=3, space="PSUM"))

        idx_sb = consts.tile([P, (TA + TB) * 8], i16)
        nc.sync.dma_start(idx_sb[:], idxd[:])
        dst_sb = consts.tile([P, TA + TB], bf16)
        nc.sync.dma_start(dst_sb[:], dstd[:])
        norm2_sb = consts.tile([P, TPC], f32)
        nc.sync.dma_start(norm2_sb[:], norm2d[:])
        norm1_sb = consts.tile([P, TPC], f32)
        nc.sync.dma_start(norm1_sb[:], norm1d[:])
        iota_sb = consts.tile([P, P], bf16)
        nc.sync.dma_start(iota_sb[:], iotad[:])
        w_sb = []
        for i in range(4):
            dcol = D if i < 3 else DOUT
            w = consts.tile([P, dcol], bf16)
            nc.sync.dma_start(w[:], wd[i][:])
            w_sb.append(w)
        out_sb = outp.tile([P, TPC * DOUT], bf16)

        qrr = [0]

        def gather_and_sel(src_tab, K, off, kt, idx_base, msgp, selp):
            """One chunk-half: gather kt slots of 128 messages + build the
            one-hot dst selector in a single 3D-broadcast is_equal."""
            msg = msgp.tile([P, kt, D], bf16, name=f"msg{idx_base}")
            for s0 in range(0, kt, GMAX):
                sw = min(GMAX, kt - s0)
                nc.gpsimd.dma_gather(
                    msg[:, s0:s0 + sw, :], src_tab,
                    idx_sb[:, (idx_base + off + s0) * 8:
                           (idx_base + off + s0 + sw) * 8],
                    sw * P, sw * P, D, queue_num=qrr[0] % 4,
                    single_packet=False)
                qrr[0] += 1
            sel = selp.tile([P, kt * P], bf16, name=f"sel{idx_base}")
            nc.vector.tensor_tensor(
                out=sel[:].rearrange("p (n r) -> p n r", n=kt),
                in0=dst_sb[:, idx_base + off:idx_base + off + kt]
                    .unsqueeze(2).broadcast_to([P, kt, P]),
                in1=iota_sb[:].unsqueeze(1).broadcast_to([P, kt, P]),
                op=mybir.AluOpType.is_equal)
            return msg, sel

        def finish_tile(layer, t, aggT):
            dcol = D if layer < 3 else DOUT
            psO = ps2.tile([P, dcol], f32, space="PSUM")
            nc.tensor.matmul(out=psO[:], lhsT=aggT,
                             rhs=w_sb[layer][:, :dcol], start=True, stop=True)
            if layer < 3:
                nc.scalar.activation(
                    out=h_next[:, t * D:(t + 1) * D], in_=psO[:],
                    func=mybir.ActivationFunctionType.Relu,
                    scale=norm2_sb[:, t:t + 1])
            else:
                nc.scalar.activation(
                    out=out_sb[:, t * DOUT:(t + 1) * DOUT], in_=psO[:],
                    func=mybir.ActivationFunctionType.Copy,
                    scale=norm1_sb[:, t:t + 1])

        h_prev = None
        for layer in range(4):
            if layer == 0:
                tabA = featrep[0:HALF, :]
                tabB = featrep[HALF:NFULL, :]
            else:
                hfirst = shp.tile([HALF, D], bf16, addr_space="Shared")
                hsecond = shp.tile([HALF, D], bf16, addr_space="Shared")
                nc.gpsimd.collective_compute(
                    "AllGather", mybir.AluOpType.bypass,
                    replica_groups=[list(range(NCORES))],
                    ins=[h_prev[0:NHL, :]], outs=[hfirst[:]])
                tabA = hfirst[:]
                tabB = hsecond[:]
            if layer < 3:
                h_next = hpool.tile([P, TPC * D], bf16)

            if layer == 0:
                # both tables ready at start: single fused phase
                for ts, oA, ka, oB, kb in chunks:
                    msgA, selA = gather_and_sel(tabA, KA, oA, ka, 0,
                                                msgpA, selpA)
                    msgB, selB = gather_and_sel(tabB, KB, oB, kb, TA,
                                                msgpB, selpB)
                    jA = jB = 0
                    for t in ts:
                        nslots = KA[t] + KB[t]
                        if nslots == 0:
                            nc.vector.memset(h_next[:, t * D:(t + 1) * D], 0.0)
                            continue
                        ps = ps1.tile([P, P], f32, space="PSUM")
                        k = 0
                        for _ in range(KA[t]):
                            nc.tensor.matmul(
                                out=ps[:], lhsT=msgA[:, jA, :],
                                rhs=selA[:, jA * P:(jA + 1) * P],
                                start=(k == 0), stop=(k == nslots - 1))
                            jA += 1
                            k += 1
                        for _ in range(KB[t]):
                            nc.tensor.matmul(
                                out=ps[:], lhsT=msgB[:, jB, :],
                                rhs=selB[:, jB * P:(jB + 1) * P],
                                start=(k == 0), stop=(k == nslots - 1))
                            jB += 1
                            k += 1
                        aggT = aggp.tile([P, P], bf16)
                        nc.vector.tensor_copy(aggT[:], ps[:])
                        finish_tile(layer, t, aggT[:])
            else:
                # phase A: gather/accumulate first-half sources; overlaps the
                # second AllGather issued right after.
                aggA = apool.tile([P, TPC * P], bf16)
                for ts, oA, ka, oB, kb in chunks:
                    if ka == 0:
                        continue
                    msgA, selA = gather_and_sel(tabA, KA, oA, ka, 0,
                                                msgpA, selpA)
                    jA = 0
                    for t in ts:
                        if KA[t] == 0:
                            continue
                        ps = ps1.tile([P, P], f32, space="PSUM")
                        for k in range(KA[t]):
                            nc.tensor.matmul(
                                out=ps[:], lhsT=msgA[:, jA, :],
                                rhs=selA[:, jA * P:(jA + 1) * P],
                                start=(k == 0), stop=(k == KA[t] - 1))
                            jA += 1
                        nc.vector.tensor_copy(
                            aggA[:, t * P:(t + 1) * P], ps[:])
                nc.gpsimd.collective_compute(
                    "AllGather", mybir.AluOpType.bypass,
                    replica_groups=[list(range(NCORES))],
                    ins=[h_prev[NHL:NPAD, :]], outs=[hsecond[:]])
                # phase B: second-half sources, then combine + linear
                for ts, oA, ka, oB, kb in chunks:
                    if kb:
                        msgB, selB = gather_and_sel(tabB, KB, oB, kb, TA,
                                                    msgpB, selpB)
                    jB = 0
                    for t in ts:
                        if KA[t] == 0 and KB[t] == 0:
                            if layer < 3:
                                nc.vector.memset(
                                    h_next[:, t * D:(t + 1) * D], 0.0)
                            else:
                                nc.vector.memset(
                                    out_sb[:, t * DOUT:(t + 1) * DOUT], 0.0)
                            continue
                        if KB[t] == 0:
                            finish_tile(layer, t, aggA[:, t * P:(t + 1) * P])
                            continue
                        ps = ps1.tile([P, P], f32, space="PSUM")
                        for k in range(KB[t]):
                            nc.tensor.matmul(
                                out=ps[:], lhsT=msgB[:, jB, :],
                                rhs=selB[:, jB * P:(jB + 1) * P],
                                start=(k == 0), stop=(k == KB[t] - 1))
                            jB += 1
                        aggT = aggp.tile([P, P], bf16)
                        if KA[t] == 0:
                            nc.vector.tensor_copy(aggT[:], ps[:])
                        else:
                            nc.vector.tensor_tensor(
                                out=aggT[:], in0=aggA[:, t * P:(t + 1) * P],
                                in1=ps[:], op=mybir.AluOpType.add)
                        finish_tile(layer, t, aggT[:])
            if layer < 3:
                bounce = dram.tile([NPAD, D], bf16)
                nc.sync.dma_start(
                    bounce[:].rearrange("(t p) f -> p t f", p=P),
                    h_next[:].rearrange("p (t f) -> p t f", t=TPC))
                h_prev = bounce
        nc.sync.dma_start(
            outd[:].rearrange("(t p) f -> p t f", p=P),
            out_sb[:].rearrange("p (t f) -> p t f", t=TPC))
    nc.compile()
    return nc


def _preprocess(features, edge_index, Ws):
    src = np.asarray(edge_index[0], dtype=np.int64)
    dst = np.asarray(edge_index[1], dtype=np.int64)
    features = np.asarray(features, dtype=np.float32)

    deg = np.bincount(dst, minlength=N).astype(np.float32)
    norm = 1.0 / np.sqrt(np.maximum(deg, 1.0))

    core = dst // NPC
    tt = (dst - core * NPC) >> 7
    rr = (dst - core * NPC) & 127
    sc = src // NPC
    sl = src - sc * NPC
    halfB = (sl >= NHL).astype(np.int64)
    gidx = sc * NHL + sl - halfB * NHL   # row within its half-table

    key = (core * 2 + halfB) * TPC + tt
    cnt = np.bincount(key, minlength=NCORES * 2 * TPC).reshape(NCORES, 2, TPC)
    KA = [int(x) for x in -(-cnt[:, 0, :].max(axis=0) // P)]
    KB = [int(x) for x in -(-cnt[:, 1, :].max(axis=0) // P)]
    TA, TB = sum(KA), sum(KB)
    baseA = P * np.concatenate([[0], np.cumsum(KA)]).astype(np.int64)
    baseB = P * np.concatenate([[0], np.cumsum(KB)]).astype(np.int64)

    # replicated layer-0 gather table: first local halves of all cores,
    # then second local halves (matches the AllGather layout for layers 1-3)
    feat_s = features * norm[:, None]
    featrep = np.zeros((NFULL, D), dtype=np.float32)
    for c in range(NCORES):
        featrep[c * NHL:c * NHL + NHL] = feat_s[c * NPC:c * NPC + NHL]
        n2 = NPC - NHL
        featrep[HALF + c * NHL:HALF + c * NHL + n2] = \
            feat_s[c * NPC + NHL:(c + 1) * NPC]
    featrep = featrep.astype(_bf16np)

    in_maps = []
    iota = np.tile(np.arange(P, dtype=np.float32), (P, 1)).astype(_bf16np)
    for c in range(NCORES):
        m = core == c
        tc_, rc_, gc_, hc_ = tt[m], rr[m], gidx[m], halfB[m]
        idx_flat = np.zeros(TA * P + TB * P, dtype=np.int16)
        dst_flat = np.full(TA * P + TB * P, -1.0, dtype=np.float32)
        for h, (K, base, off) in enumerate(
                [(KA, baseA, 0), (KB, baseB, TA * P)]):
            sel = hc_ == h
            th, rh, gh = tc_[sel], rc_[sel], gc_[sel]
            o = np.lexsort((gh, th))
            th, rh, gh = th[o], rh[o], gh[o]
            per_tile = np.bincount(th, minlength=TPC)
            run_first = np.concatenate([[0], np.cumsum(per_tile)])[:-1]
            rank = np.arange(len(th)) - run_first[th]
            pos = off + base[th] + rank
            idx_flat[pos] = gh.astype(np.int16)
            dst_flat[pos] = rh.astype(np.float32)
        # wrap-16 index layout, replicated across the 8 gpsimd cores
        wrapped = np.tile(idx_flat.reshape(-1, 16).T, (8, 1))
        dst_cols = dst_flat.reshape(TA + TB, P).T.astype(_bf16np)

        nloc = np.zeros(NPAD, dtype=np.float32)
        nloc[:NPC] = norm[c * NPC:(c + 1) * NPC]
        in_maps.append({
            "featrep": featrep,
            "idxd": np.ascontiguousarray(wrapped),
            "dstd": np.ascontiguousarray(dst_cols),
            "norm2d": np.ascontiguousarray(
                (nloc * nloc).reshape(TPC, P).T),
            "norm1d": np.ascontiguousarray(nloc.reshape(TPC, P).T),
            "iotad": iota,
            "w0": Ws[0].astype(_bf16np), "w1": Ws[1].astype(_bf16np),
            "w2": Ws[2].astype(_bf16np), "w3": Ws[3].astype(_bf16np),
        })
    return KA, KB, in_maps


class _Runner:
    """Caches the jitted shard_map executable for a compiled Bass module and
    provides one-shot runs (host in / host out) plus a chained-dispatch
    benchmark that amortizes the PJRT round-trip latency."""

    def __init__(self, nc):
        import jax
        from jax.sharding import Mesh, PartitionSpec, NamedSharding
        from jax.experimental.shard_map import shard_map
        from concourse.bass2jax import (
            _bass_exec_p, install_neuronx_cc_hook, partition_id_tensor)
        install_neuronx_cc_hook()
        self.jax = jax
        self.nc = nc
        partition_name = (nc.partition_id_tensor.name
                          if nc.partition_id_tensor else None)
        in_names, out_names, out_avals, zero_outs = [], [], [], []
        for alloc in nc.m.functions[0].allocations:
            if not isinstance(alloc, mybir.MemoryLocationSet):
                continue
            name = alloc.memorylocations[0].name
            if alloc.kind == "ExternalInput":
                if name != partition_name:
                    in_names.append(name)
            elif alloc.kind == "ExternalOutput":
                out_names.append(name)
                shape = tuple(alloc.tensor_shape)
                dtype = mybir.dt.np(alloc.dtype)
                out_avals.append(jax.core.ShapedArray(shape, dtype))
                zero_outs.append(np.zeros(shape, dtype))
        n_params = len(in_names)
        n_outs = len(out_avals)
        all_names = list(in_names) + list(out_names)
        if partition_name is not None:
            all_names.append(partition_name)
        donate = tuple(range(n_params, n_params + n_outs))
        self.in_names = in_names
        self.out_names = out_names
        self.out_avals = out_avals
        self.zero_outs = zero_outs
        self.n_params = n_params

        def _body(*args):
            operands = list(args)
            if partition_name is not None:
                operands.append(partition_id_tensor())
            outs = _bass_exec_p.bind(
                *operands, out_avals=tuple(out_avals),
                in_names=tuple(all_names), out_names=tuple(out_names),
                lowering_input_output_aliases=(),
                sim_require_finite=True, sim_require_nnan=True, nc=nc)
            return tuple(outs)

        devices = jax.devices()[:NCORES]
        mesh = Mesh(np.asarray(devices), ("core",))
        self.sharding = NamedSharding(mesh, PartitionSpec("core"))
        self.sharded = jax.jit(
            shard_map(_body, mesh=mesh,
                      in_specs=(PartitionSpec("core"),) * (n_params + n_outs),
                      out_specs=(PartitionSpec("core"),) * n_outs,
                      check_rep=False),
            donate_argnums=donate, keep_unused=True)
        self._staged = None

    def _concat_inputs(self, in_maps):
        return [np.concatenate([np.asarray(in_maps[c][n])
                                for c in range(NCORES)], axis=0)
                for n in self.in_names]

    def _fresh_zeros(self):
        return [np.zeros((NCORES * z.shape[0], *z.shape[1:]), z.dtype)
                for z in self.zero_outs]

    def run(self, in_maps):
        out_arrs = self.sharded(*self._concat_inputs(in_maps),
                                *self._fresh_zeros())
        return [
            {n: np.asarray(out_arrs[i]).reshape(
                NCORES, *self.out_avals[i].shape)[c]
             for i, n in enumerate(self.out_names)}
            for c in range(NCORES)]

    def stage(self, in_maps):
        jax = self.jax
        dev_in = [jax.device_put(x, self.sharding)
                  for x in self._concat_inputs(in_maps)]
        jax.block_until_ready(dev_in)
        self._staged = dev_in
        return dev_in

    def bench(self, in_maps, iters=50):
        """Best per-run wall time over chained executions: inputs staged on
        device, each run's output buffer donated to the next run (serialized
        on device by the data dependency), one sync at the end."""
        import time
        jax = self.jax
        dev_in = self.stage(in_maps)
        outs = [jax.device_put(z, self.sharding) for z in self._fresh_zeros()]
        jax.block_until_ready(outs)
        outs = self.sharded(*dev_in, *outs)
        jax.block_until_ready(outs)
        best = float("inf")
        for _ in range(3):
            t0 = time.time()
            cur = outs
            for _ in range(iters):
                cur = self.sharded(*dev_in, *cur)
            jax.block_until_ready(cur)
            best = min(best, (time.time() - t0) / iters)
            outs = cur
        return best


def kernel(features, edge_index, W0, W1, W2, W3):
    global _last_in_maps
    Ws = [np.ascontiguousarray(np.asarray(w, dtype=np.float32))
          for w in (W0, W1, W2, W3)]
    KA, KB, in_maps = _preprocess(features, edge_index, Ws)
    key = (tuple(KA), tuple(KB))
    if key not in _cache:
        _cache[key] = _build(KA, KB)
    nc = _cache[key]
    if key not in _runner_cache:
        _runner_cache[key] = _Runner(nc)
    runner = _runner_cache[key]
    _last_in_maps = in_maps
    res = runner.run(in_maps)
    out = np.concatenate(
        [res[c]["out"][:NPC].astype(np.float32) for c in range(NCORES)],
        axis=0)
    return out


def bench_ns(iters=50):
    """Benchmark the last-run configuration; returns best per-run ns."""
    assert _runner_cache and _last_in_maps is not None
    runner = next(iter(_runner_cache.values()))
    return int(runner.bench(_last_in_maps, iters=iters) * 1e9)
